# revision 32
# baseline (speedup 1.0000x reference)
"""Trainium2 Bass kernel for nn_Decoder (mapping MLP + hard-LSTM scan + out proj).

Self-contained: takes FULL inputs (as produced by setup_inputs), shards batch
across 8 NeuronCores, runs a Bass/Tile kernel via run_bass_kernel_spmd, and
gathers the full [T, K, B, C] output.

Layout per core (B' = B/8 = 512 batch elems):
  rows = k*B' + b  (20 "rtiles" of 512 rows each, one per k)
  h, c state: [H=128 partitions, 512 rows] bf16 tiles, one pair per rtile.

Per step & rtile (engine assignment tuned against the TRN2 cost model):
  PE   : 4 W_ih-[x;1] preloads + 4 W_hh matmuls fill two PSUM bank-pairs
         [i|g] and [f|o]; the out-projection runs TRANSPOSED (h 128-batch
         chunk as stationary, W_outT as moving) so it costs ~2 columns
         instead of 512, accumulating [128 batch, 2] slots in a PSUM bank.
  ACT  : one merged relu evac of [f|o] -> a_fo bf16; PSUM out-bank evac
         every 64 rtile-steps.
  DVE  : OP_T3 t = hs(i)*ht(g) straight from both PSUM banks;
         OP_UC u = min(a_f,1)*c; OP_H h = min(a_o,1)*ht(c) (lagged 1 rtile
         so the GPSIMD add can complete without stalling the DVE stream).
  POOL : c = u + t (native tensor add on the otherwise idle GPSIMD engine).
"""
import os
import sys

sys.path.insert(0, "/opt/trn_rl_repo")

import numpy as np
import ml_dtypes
from contextlib import ExitStack

import concourse.bass as bass
import concourse.tile as tile
from concourse import mybir, bacc
import concourse.dve_ops as _dve_ops_mod
from concourse.dve_ops import (
    DveOp, OPS, CUSTOM_DVE_SPECS, _CUSTOM_DVE_ROW_BASE, get_dve_sub_opcode,
    _COMPILE_CACHE,
)
from concourse.dve_spec import (
    Spec, Src0, Src1, C0, C1, C2, Zero, One, maxx, minn, relu, lower, _has_src1,
)
from concourse.dve_uop import (
    DveOpSpec, UopConfig, UopDpConfig, InpSel, OutSel, OutPath, AluInp,
    AluOp as UAluOp, DelayInp, Trigger,
)
from concourse.bass_utils import run_bass_kernel_spmd

FP32 = mybir.dt.float32
BF16 = mybir.dt.bfloat16
AF = mybir.ActivationFunctionType

# Full-problem config (hardcoded; the harness always calls with these shapes).
T_FULL, K_FULL, B_FULL, C_DIM, H_DIM, MH_DIM, N_CORES = 20, 20, 4096, 2, 128, 64, 8


# ---------------------------------------------------------------- custom ops
def _register_op(name, spec):
    for op in OPS:
        if op.name == name:
            return op
    shas = {}
    for ver in ("v3", "v4"):
        tmp = DveOpSpec(name=name, opcode=0, uops=lower(spec, ver=ver),
                        rd1_en=_has_src1(spec))
        shas[ver] = tmp.sha(ver)
    op = DveOp(name, spec, subdim=False, uops_sha=shas)
    OPS.append(op)
    CUSTOM_DVE_SPECS[name] = spec
    _dve_ops_mod._SUB_OPCODE_FOR_NAME[name] = _CUSTOM_DVE_ROW_BASE + len(OPS) - 1
    assert _dve_ops_mod._SUB_OPCODE_FOR_NAME[name] < 0x20
    return op


def _uop_h_2x():
    """2X_1PORT program for ANT_LSTM_H: elements lo/hi of min(Src0,1) *
    max(min(Src1,1), C2). Element 0 in blocks 0-3 (result rides delay lane 0
    to the output), element 1 in blocks 4-7 (result = block 7 ALU out).

    Input lanes (lane k feeds block 0's delay chain k-1):
      1:SRC_0->d0  2:ONE->d1  3:SRC_1->d2  4:CONST_2->d3
      5:SRC_0_HI->d4  6:SRC_1_HI->d5
    """
    u = UopConfig()
    for lane, src in ((1, InpSel.SRC_0), (2, InpSel.ONE_F32),
                      (3, InpSel.SRC_1), (4, InpSel.CONST_2),
                      (5, InpSel.SRC_0_HI), (6, InpSel.SRC_1_HI)):
        u.enable_input(src, lane)
    u.require_inp0 = 1
    u.require_inp1 = 1
    u.trigger = (Trigger.SRC_TENSOR_DONE, Trigger.NONE, Trigger.NONE)
    dp = u.datapath_config
    dp[0].enable_alu(UAluOp.MIN, AluInp.PREV_DELAY_2, AluInp.PREV_DELAY_1)
    dp[0].pass_through_delay(0, 1, 3, 4, 5)
    dp[1].enable_alu(UAluOp.MAX, AluInp.PREV_ALU_OUT, AluInp.PREV_DELAY_3)
    dp[1].pass_through_delay(0, 1, 3, 4, 5)
    dp[2].enable_alu(UAluOp.MIN, AluInp.PREV_DELAY_0, AluInp.PREV_DELAY_1)
    dp[2].enable_delay_from_src(DelayInp.PREV_ALU_OUT, 0)   # capture ht_lo
    dp[2].pass_through_delay(1, 3, 4, 5)
    dp[3].enable_alu(UAluOp.MULTIPLY, AluInp.PREV_ALU_OUT, AluInp.PREV_DELAY_0)
    dp[3].pass_through_delay(1, 3, 4, 5)
    dp[4].enable_alu(UAluOp.MIN, AluInp.PREV_DELAY_5, AluInp.PREV_DELAY_1)
    dp[4].enable_delay_from_src(DelayInp.PREV_ALU_OUT, 0)   # capture r_lo
    dp[4].pass_through_delay(1, 3, 4)
    dp[5].enable_alu(UAluOp.MAX, AluInp.PREV_ALU_OUT, AluInp.PREV_DELAY_3)
    dp[5].pass_through_delay(0, 1, 4)
    dp[6].enable_alu(UAluOp.MIN, AluInp.PREV_DELAY_4, AluInp.PREV_DELAY_1)
    dp[6].enable_delay_from_src(DelayInp.PREV_ALU_OUT, 2)   # capture ht_hi
    dp[6].pass_through_delay(0)
    dp[7].enable_alu(UAluOp.MULTIPLY, AluInp.PREV_ALU_OUT, AluInp.PREV_DELAY_2)
    dp[7].pass_through_delay(0)
    u.enable_output(OutSel.DELAY_0, OutPath.WR0_LO)
    u.enable_output(OutSel.ALU_OUT, OutPath.WR0_HI)
    return [u]


def _uop_uc_2x():
    """2X_1PORT program for ANT_LSTM_UC: lo/hi of min(Src0,1) * Src1.
    Lanes: 1:SRC_0->d0 2:ONE->d1 3:SRC_1->d2 4:SRC_0_HI->d3 5:SRC_1_HI->d4."""
    u = UopConfig()
    for lane, src in ((1, InpSel.SRC_0), (2, InpSel.ONE_F32),
                      (3, InpSel.SRC_1), (4, InpSel.SRC_0_HI),
                      (5, InpSel.SRC_1_HI)):
        u.enable_input(src, lane)
    u.require_inp0 = 1
    u.require_inp1 = 1
    u.trigger = (Trigger.SRC_TENSOR_DONE, Trigger.NONE, Trigger.NONE)
    dp = u.datapath_config
    dp[0].enable_alu(UAluOp.MIN, AluInp.PREV_DELAY_0, AluInp.PREV_DELAY_1)
    dp[0].pass_through_delay(1, 2, 3, 4)
    dp[1].enable_alu(UAluOp.MULTIPLY, AluInp.PREV_ALU_OUT, AluInp.PREV_DELAY_2)
    dp[1].pass_through_delay(1, 3, 4)
    dp[2].enable_alu(UAluOp.MIN, AluInp.PREV_DELAY_3, AluInp.PREV_DELAY_1)
    dp[2].enable_delay_from_src(DelayInp.PREV_ALU_OUT, 0)   # capture r_lo
    dp[2].pass_through_delay(4)
    dp[3].enable_alu(UAluOp.MULTIPLY, AluInp.PREV_ALU_OUT, AluInp.PREV_DELAY_4)
    dp[3].pass_through_delay(0)
    for b in range(4, 8):
        dp[b].pass_through_alu()
        dp[b].pass_through_delay(0)
    u.enable_output(OutSel.DELAY_0, OutPath.WR0_LO)
    u.enable_output(OutSel.ALU_OUT, OutPath.WR0_HI)
    return [u]


def _enable_2x(op, uops_2x):
    """Pre-populate the DveOp compile cache with a spec carrying a 2X_1PORT
    uop program so dve_table_for_ops writes the mode entries. Instructions
    opt in per call site via ins.perf_max = 1 (the engine still falls back
    to 1x at runtime when operands are not packed 2-byte)."""
    for ver in ("v3", "v4"):
        spec = DveOpSpec(
            name=op.name,
            opcode=get_dve_sub_opcode(op.name),
            uops=lower(op.spec, ver=ver),
            rd1_en=_has_src1(op.spec),
            uops_2x=uops_2x,
            perf_max=1,
        )
        _COMPILE_CACHE[(op.name, ver)] = spec


def _hs(x, s0, s1):
    return np.minimum(np.maximum(x * s0 + s1, 0.0), 1.0)


def _ht(x, lo):
    return np.maximum(np.minimum(x, 1.0), lo)


# u = min(a_f, 1) * c ; a_f = relu(f') from ACT
OP_UC = _register_op(
    "ANT_LSTM_UC",
    Spec(body=minn(Src0, One) * Src1,
         reference=lambda in0, in1, s0, s1, imm2: np.minimum(in0, 1.0) * in1),
)
# u = hs(f') * c with f' read straight from PSUM (relu+min in-op)
OP_UF = _register_op(
    "ANT_LSTM_UF",
    Spec(body=minn(relu(Src0), One) * Src1,
         reference=lambda in0, in1, s0, s1, imm2:
             np.minimum(np.maximum(in0, 0.0), 1.0) * in1),
)
# h = min(a_o,1)*clip(c, -1, 1); a_o = relu(o') from ACT; imm2=-1.
# Also used for t = min(a_i,1)*ht(g) with in1 = the raw g-gate PSUM bank
# (DVE may read at most one non-scalar operand from PSUM).
OP_H = _register_op(
    "ANT_LSTM_H",
    Spec(body=minn(Src0, One) * maxx(minn(Src1, One), C2),
         reference=lambda in0, in1, s0, s1, imm2:
             np.minimum(in0, 1.0) * np.maximum(np.minimum(in1, 1.0), imm2)),
)
# leaky_relu(psum + bm1) = max(y, 0.01*y), y = Src0 + C1; imm2 = slope
OP_LRELU = _register_op(
    "ANT_LRELU",
    Spec(body=maxx(Src0 + C1, (Src0 + C1) * C2),
         reference=lambda in0, in1, s0, s1, imm2:
             np.maximum(in0 + s1, (in0 + s1) * imm2)),
)

_enable_2x(OP_H, _uop_h_2x())
_enable_2x(OP_UC, _uop_uc_2x())


# ---------------------------------------------------------------- bass build
def build_nc(T, K, BP, use_pack=True, repeat=1, tmp_bufs=4,
             c_add_eng="vector", out_lag=4, h_lag=2, evac_mode="fo_merged",
             pair_mode="ig_fo", out_evac_eng="scalar", out_pos="late",
             **_unused):
    """Build the per-core Bass program. BP = per-core batch (must be 512).

    PSUM banks (8 x 512 fp32): pa = [i | g] x2 bufs (DVE-read), pb = [f | o]
    x1 buf (ACT-evac'd early), po = out-projection slot banks x2 bufs.
    Biases + hardsigmoid pre-scale folded into the augmented W_ih-[x;1]
    preload on the host: i/f/o psum arrive as (raw+b)/6+0.5, g as raw+b.
    """
    H, MH, C = H_DIM, MH_DIM, C_DIM
    CA = C + 1        # augmented x rows (x0, x1, 1)
    FD = BP           # free dim of every tile
    PSB = 512         # PSUM bank stride in fp32 elems
    RT = K            # rtiles per step
    assert FD == PSB, "layout assumes BP == 512"
    nc = bacc.Bacc("TRN2", target_bir_lowering=False, debug=False)

    phT_e = nc.declare_dram_parameter("phT", [H, K * BP], BF16, isOutput=False)
    xh_e = nc.declare_dram_parameter("xh", [CA, T * BP], BF16, isOutput=False)
    whhT_e = nc.declare_dram_parameter("whhT", [H, 4 * H], BF16, isOutput=False)
    wihT_e = nc.declare_dram_parameter("wihT", [CA, 4 * H], BF16, isOutput=False)
    woutT_e = nc.declare_dram_parameter("woutT", [H, C], BF16, isOutput=False)
    wm1T_e = nc.declare_dram_parameter("wm1T", [H, MH], BF16, isOutput=False)
    wm2T_e = nc.declare_dram_parameter("wm2T", [MH, H], BF16, isOutput=False)
    biasp_e = nc.declare_dram_parameter("biasp", [H, 2], FP32, isOutput=False)
    # out: [128 batch-chunk partitions, T*K*4chunks*2coords] fp32
    NOUT = T * K * 4 * C
    out_e = nc.declare_dram_parameter("out", [128, NOUT], FP32, isOutput=True)

    with tile.TileContext(nc) as tc:
        with ExitStack() as ctx:
            wts = ctx.enter_context(tc.tile_pool(name="wts", bufs=1))
            big = ctx.enter_context(tc.tile_pool(name="big", bufs=1))
            st = ctx.enter_context(tc.tile_pool(name="st", bufs=1))
            tmp = ctx.enter_context(tc.tile_pool(name="tmp", bufs=tmp_bufs))
            psA = ctx.enter_context(tc.tile_pool(name="psA", bufs=2, space="PSUM"))
            psB = ctx.enter_context(tc.tile_pool(name="psB", bufs=2, space="PSUM"))

            # ---- weights / constants into SBUF
            whhT = wts.tile([H, 4 * H], BF16, tag="whhT")
            nc.sync.dma_start(whhT[:], whhT_e[:])
            woutT = wts.tile([H, C], BF16, tag="woutT")
            nc.sync.dma_start(woutT[:], woutT_e[:])
            wm1T = wts.tile([H, MH], BF16, tag="wm1T")
            nc.sync.dma_start(wm1T[:], wm1T_e[:])
            wm2T = wts.tile([MH, H], BF16, tag="wm2T")
            nc.sync.dma_start(wm2T[:], wm2T_e[:])
            biasp = wts.tile([H, 2], FP32, tag="biasp")
            nc.sync.dma_start(biasp[:], biasp_e[:])

            npack = 4 if use_pack else 1
            nprows = 32 * (npack - 1) + CA
            wih = wts.tile([nprows, 4 * H], BF16, tag="wih")
            xrep = wts.tile([nprows, T * BP], BF16, tag="xrep")
            for r in range(npack):
                nc.sync.dma_start(wih[32 * r:32 * r + CA, :], wihT_e[:])
                nc.sync.dma_start(xrep[32 * r:32 * r + CA, :], xh_e[:])

            phT = big.tile([H, K * BP], BF16, tag="phT")
            nc.sync.dma_start(phT[:], phT_e[:])

            # SBUF accumulation buffer for the final output
            out_sb = big.tile([128, NOUT], FP32, tag="out_sb")

            # ---- persistent state tiles
            h_t = [st.tile([H, FD], BF16, name=f"h{j}", tag=f"h{j}")
                   for j in range(RT)]
            c_t = [st.tile([H, FD], BF16, name=f"c{j}", tag=f"c{j}")
                   for j in range(RT)]

            # ---- mapping MLP -> h0
            for j in range(RT):
                pa = psA.tile([H, 2 * PSB], FP32, tag="pa")
                nc.tensor.matmul(pa[0:MH, 0:FD], wm1T[:, 0:MH],
                                 phT[:, j * FD:(j + 1) * FD],
                                 start=True, stop=True)
                a1 = tmp.tile([MH, FD], BF16, tag="a1")
                nc.vector._custom_dve(OP_LRELU, out=a1[:], in0=pa[0:MH, 0:FD],
                                      s1=biasp[0:MH, 0:1], imm2=0.01)
                nc.tensor.matmul(pa[0:H, PSB:PSB + FD], wm2T[:, 0:H], a1[:],
                                 start=True, stop=True)
                nc.scalar.activation(h_t[j][:], pa[0:H, PSB:PSB + FD],
                                     AF.Identity, bias=biasp[:, 1:2], scale=1.0)

            # gate chunk offsets in whhT / wih cols: i=0, f=1, g=2, o=3
            CH = {"i": 0, "f": 1, "g": 2, "o": 3}

            def gcol(name):
                m = CH[name]
                return slice(m * H, (m + 1) * H)

            def gates_mm(ps, xcols, j, names):
                # W_ih-[x;1] preloads (contract=3, row-packed), then W_hh accum
                for r, gname in enumerate(names):
                    rr = (CH[gname] if use_pack else 0)
                    sl = slice(0, FD) if r == 0 else slice(PSB, PSB + FD)
                    nc.tensor.matmul(
                        ps[:, sl],
                        wih[32 * rr:32 * rr + CA, gcol(gname)],
                        xrep[32 * rr:32 * rr + CA, xcols],
                        start=True, stop=False,
                        tile_position=(32 * rr, 0) if use_pack else None,
                    )
                for r, gname in enumerate(names):
                    sl = slice(0, FD) if r == 0 else slice(PSB, PSB + FD)
                    nc.tensor.matmul(ps[:, sl], whhT[:, gcol(gname)],
                                     h_t[j][:], start=False, stop=True)

            # ---- flat software-pipelined main loop over s = tl*RT + j.
            # The [i|g] pa-group runs ONE rtile ahead of the [f|o] pb-group so
            # the PE has ~1.7us of queued work inside the a_fo ACT-evac window
            # (pb is single-buffered; its bank is reusable only after a_fo).
            S = T * repeat * RT
            pa_tiles = {}   # s -> (pa_tile, a_i_tile)
            afo_tiles = {}  # s -> a_fo_tile
            pend_h = []     # s queued for OP_H (lagged h_lag rtiles)
            pend_out = []   # s queued for out-proj (lagged out_lag rtiles)

            def s_tj(s):
                return (s // RT) % T, s % RT

            def emit_pa(s):
                t, j = s_tj(s)
                if pair_mode == "ig_fo":
                    pa = psA.tile([H, 2 * PSB], FP32, tag="pa")  # [i | g]
                    gates_mm(pa, slice(t * BP, (t + 1) * BP), j, "ig")
                    a_i = tmp.tile([H, FD], BF16, tag="ai")
                    nc.scalar.activation(a_i[:], pa[:, 0:FD], AF.Relu,
                                         bias=0.0, scale=1.0)
                    pa_tiles[s] = (pa, a_i[:])
                else:  # gf_io: both halves consumed by DVE straight from psum
                    pa = psA.tile([H, 2 * PSB], FP32, tag="pa")  # [g | f]
                    gates_mm(pa, slice(t * BP, (t + 1) * BP), j, "gf")
                    pa_tiles[s] = (pa, None)

            def emit_h(s):
                t, j = s_tj(s)
                a_o_ap = afo_tiles.pop(s)
                r = nc.vector._custom_dve(OP_H, out=h_t[j][:], in0=a_o_ap,
                                          in1=c_t[j][:], imm2=-1.0)
                r.ins.perf_max = 1   # all-bf16 SBUF -> 2X_1PORT eligible

            def emit_out(s, host_pa, col_off=0):
                # Transposed out-projection: h 128-batch chunk is stationary,
                # woutT moving -> [128 batch, 2] psum slots. The slots land in
                # the DEAD tail of `host_pa`'s g-bank (already consumed by
                # OP_H-as-t), so no dedicated psum bank is needed and psB can
                # double-buffer. Evac'd to out_sb immediately (8 fp32).
                t, j = s_tj(s)
                sm = s % (T * RT)
                gend = 2 * PSB if pair_mode == "ig_fo" else PSB
                base = gend - 8 - col_off
                for q in range(4):
                    nc.tensor.matmul(host_pa[:, base + C * q:base + C * (q + 1)],
                                     h_t[j][:, 128 * q:128 * (q + 1)],
                                     woutT[:, 0:C], start=True, stop=True)
                if out_evac_eng == "scalar":
                    nc.scalar.activation(out_sb[:, sm * 8:sm * 8 + 8],
                                         host_pa[:, base:base + 8], AF.Copy,
                                         bias=0.0, scale=1.0)
                else:
                    nc.vector.tensor_scalar_add(out_sb[:, sm * 8:sm * 8 + 8],
                                                host_pa[:, base:base + 8], 0.0)

            emit_pa(0)
            prev_pa = None
            for s in range(S):
                t, j = s_tj(s)
                if out_pos == "early" and prev_pa is not None \
                        and len(pend_out) > out_lag:
                    emit_out(pend_out.pop(0), prev_pa)
                pb = psB.tile([H, 2 * PSB], FP32, tag="pb")
                if pair_mode == "ig_fo":
                    gates_mm(pb, slice(t * BP, (t + 1) * BP), j, "fo")
                    branch = (("o_only" if s % 2 else "fo_merged")
                              if evac_mode == "alt" else evac_mode)
                    if branch == "o_only":
                        a_o = tmp.tile([H, FD], BF16, tag="ao")
                        nc.scalar.activation(a_o[:], pb[:, PSB:PSB + FD],
                                             AF.Relu, bias=0.0, scale=1.0)
                        afo_tiles[s] = a_o[:]
                        if t > 0:
                            u_d = tmp.tile([H, FD], BF16, tag="uu")
                            nc.vector._custom_dve(OP_UF, out=u_d[:],
                                                  in0=pb[:, 0:FD],
                                                  in1=c_t[j][:])
                    else:
                        a_fo = tmp.tile([H, 2 * PSB], BF16, tag="afo")
                        nc.scalar.activation(a_fo[:], pb[:, 0:2 * PSB],
                                             AF.Relu, bias=0.0, scale=1.0)
                        afo_tiles[s] = a_fo[:, PSB:PSB + FD]
                        if t > 0:
                            u_d = tmp.tile([H, FD], BF16, tag="uu")
                            r = nc.vector._custom_dve(OP_UC, out=u_d[:],
                                                      in0=a_fo[:, 0:FD],
                                                      in1=c_t[j][:])
                            r.ins.perf_max = 1
                else:  # gf_io: pb = [i | o], one merged ACT evac covers both
                    gates_mm(pb, slice(t * BP, (t + 1) * BP), j, "io")
                    a_io = tmp.tile([H, 2 * PSB], BF16, tag="afo")
                    nc.scalar.activation(a_io[:], pb[:, 0:2 * PSB], AF.Relu,
                                         bias=0.0, scale=1.0)
                    afo_tiles[s] = a_io[:, PSB:PSB + FD]   # a_o
                if s + 1 < S:
                    emit_pa(s + 1)   # prefetched pa pair for the next rtile

                # DVE: t = min(a_i,1)*ht(g); g raw+b read from psum.
                # The g-bank tail (last 8 cols) doubles as the out-proj slot
                # region after this read (WAR-sem'd by the tile pool).
                pa, a_i = pa_tiles.pop(s)
                if pair_mode == "gf_io":
                    a_i = a_io[:, 0:FD]
                    g_ap = pa[:, 0:FD]
                    if t > 0:
                        u_d = tmp.tile([H, FD], BF16, tag="uu")
                        nc.vector._custom_dve(OP_UF, out=u_d[:],
                                              in0=pa[:, PSB:PSB + FD],
                                              in1=c_t[j][:])
                else:
                    g_ap = pa[:, PSB:PSB + FD]
                if t == 0:
                    t_dst = c_t[j]      # c0 = 0 -> c1 = t
                else:
                    t_dst = tmp.tile([H, FD], BF16, tag="tt")
                nc.vector._custom_dve(OP_H, out=t_dst[:], in0=a_i,
                                      in1=g_ap, imm2=-1.0)
                # lagged OP_H (waits out the GPSIMD add latency off-stream)
                if len(pend_h) >= h_lag:
                    emit_h(pend_h.pop(0))
                if t > 0:
                    add_e = getattr(nc, c_add_eng)
                    add_e.tensor_add(c_t[j][:], u_d[:], t_dst[:])
                pend_h.append(s)
                if j == RT - 1:      # step boundary: drain so h is final
                    while pend_h:
                        emit_h(pend_h.pop(0))

                pend_out.append(s)
                if out_pos == "late" and len(pend_out) > out_lag:
                    emit_out(pend_out.pop(0), pa)
                prev_pa = pa
            last_pa = pa
            for k in range(len(pend_out)):
                emit_out(pend_out[k], last_pa, col_off=8 * (k + 1))
            pend_out.clear()

            nc.sync.dma_start(out_e[:], out_sb[:])

    nc.finalize()
    return nc


# ---------------------------------------------------------------- host side
def _bf16(x):
    return np.ascontiguousarray(x, dtype=np.float32).astype(ml_dtypes.bfloat16)


def prep_core_inputs(inputs, core, T, K, BP):
    H, MH, C = H_DIM, MH_DIM, C_DIM
    b0 = core * BP
    ph = np.asarray(inputs["pred_lstm_hidden"], np.float32)[:, b0:b0 + BP, :]
    phT = ph.transpose(2, 0, 1).reshape(H, K * BP)
    idx = np.concatenate([[0], np.arange(T - 1)])
    obs = np.asarray(inputs["obs_traj_rel"], np.float32)
    xs = obs[idx][:, b0:b0 + BP, :C]
    xh = xs.transpose(2, 0, 1).reshape(C, T * BP)
    xh = np.concatenate([xh, np.ones((1, T * BP), np.float32)], axis=0)
    bsum = (np.asarray(inputs["b_ih"], np.float32)
            + np.asarray(inputs["b_hh"], np.float32))
    # per-gate-chunk scale and bias folded into W_hh / W_ih / the x=1 row:
    #   i, f, o chunks: psum = (raw + b)/6 + 0.5 ; g chunk: psum = raw + b
    scale = np.ones(4 * H, np.float32) / 6.0
    scale[2 * H:3 * H] = 1.0
    bias_row = bsum * scale
    bias_row[0:2 * H] += 0.5
    bias_row[3 * H:4 * H] += 0.5
    whh_s = np.asarray(inputs["W_hh"], np.float32) * scale[:, None]
    wih_s = np.asarray(inputs["W_ih"], np.float32) * scale[:, None]
    wih_aug = np.concatenate([wih_s.T, bias_row[None, :]], axis=0)  # [3, 4H]
    biasp = np.zeros((H, 2), np.float32)
    biasp[0:MH, 0] = np.asarray(inputs["bm1"], np.float32)
    biasp[:, 1] = np.asarray(inputs["bm2"], np.float32)
    return {
        "phT": _bf16(phT),
        "xh": _bf16(xh),
        "whhT": _bf16(whh_s.T),
        "wihT": _bf16(wih_aug),
        "woutT": _bf16(np.asarray(inputs["W_out"], np.float32).T),
        "wm1T": _bf16(np.asarray(inputs["Wm1"], np.float32).T),
        "wm2T": _bf16(np.asarray(inputs["Wm2"], np.float32).T),
        "biasp": biasp,
    }


_NC_CACHE = {}


def _get_nc(T, K, BP):
    key = (T, K, BP)
    if key not in _NC_CACHE:
        _NC_CACHE[key] = build_nc(T, K, BP)
    return _NC_CACHE[key]


def kernel(**inputs) -> np.ndarray:
    T, K, B, C = T_FULL, K_FULL, B_FULL, C_DIM
    BP = B // N_CORES
    nc = _get_nc(T, K, BP)
    in_maps = [prep_core_inputs(inputs, c, T, K, BP) for c in range(N_CORES)]
    trace = bool(int(os.environ.get("KERNEL_TRACE", "0")))
    res = run_bass_kernel_spmd(nc, in_maps, list(range(N_CORES)), trace=trace)
    if trace:
        kernel.last_exec_time_ns = res.exec_time_ns
        kernel.last_results = res
    # per-core out: [128, T*K*4*C] -> [T, K, BP, C]
    parts = []
    for c in range(N_CORES):
        arr = res.results[c]["out"].reshape(128, T, K, 4, C)
        parts.append(arr.transpose(1, 2, 3, 0, 4).reshape(T, K, BP, C))
    full = np.concatenate(parts, axis=2)  # [T, K, B, C]
    b_out = np.asarray(inputs["b_out"], np.float32)
    return np.ascontiguousarray(full + b_out, dtype=np.float32)


# revision 38
# speedup vs baseline: 1.7160x; 1.7160x over previous
"""Trainium2 Bass kernel for nn_Decoder (mapping MLP + hard-LSTM scan + out proj).

Self-contained: takes FULL inputs (as produced by setup_inputs), shards batch
across 8 NeuronCores, runs a Bass/Tile kernel via run_bass_kernel_spmd, and
gathers the full [T, K, B, C] output.

Layout per core (B' = B/8 = 512 batch elems):
  rows = k*B' + b  (20 "rtiles" of 512 rows each, one per k)
  h, c state: [H=128 partitions, 512 rows] bf16 tiles, one pair per rtile.

Per step & rtile (engine assignment tuned against the TRN2 cost model):
  PE   : 4 W_ih-[x;1] preloads + 4 W_hh matmuls fill two PSUM bank-pairs
         [i|g] and [f|o]; the out-projection runs TRANSPOSED (h 128-batch
         chunk as stationary, W_outT as moving) so it costs ~2 columns
         instead of 512, accumulating [128 batch, 2] slots in a PSUM bank.
  ACT  : one merged relu evac of [f|o] -> a_fo bf16; PSUM out-bank evac
         every 64 rtile-steps.
  DVE  : OP_T3 t = hs(i)*ht(g) straight from both PSUM banks;
         OP_UC u = min(a_f,1)*c; OP_H h = min(a_o,1)*ht(c) (lagged 1 rtile
         so the GPSIMD add can complete without stalling the DVE stream).
  POOL : c = u + t (native tensor add on the otherwise idle GPSIMD engine).
"""
import os
import sys

sys.path.insert(0, "/opt/trn_rl_repo")

import numpy as np
import ml_dtypes
from contextlib import ExitStack

import concourse.bass as bass
import concourse.tile as tile
from concourse import mybir, bacc
import concourse.dve_ops as _dve_ops_mod
from concourse.dve_ops import (
    DveOp, OPS, CUSTOM_DVE_SPECS, _CUSTOM_DVE_ROW_BASE, get_dve_sub_opcode,
    _COMPILE_CACHE,
)
from concourse.dve_spec import (
    Spec, Src0, Src1, C0, C1, C2, Zero, One, maxx, minn, relu, lower, _has_src1,
)
from concourse.dve_uop import (
    DveOpSpec, UopConfig, UopDpConfig, InpSel, OutSel, OutPath, AluInp,
    AluOp as UAluOp, DelayInp, Trigger,
)
from concourse.bass_utils import run_bass_kernel_spmd

FP32 = mybir.dt.float32
BF16 = mybir.dt.bfloat16
AF = mybir.ActivationFunctionType

# Full-problem config (hardcoded; the harness always calls with these shapes).
T_FULL, K_FULL, B_FULL, C_DIM, H_DIM, MH_DIM, N_CORES = 20, 20, 4096, 2, 128, 64, 8


# ---------------------------------------------------------------- custom ops
def _register_op(name, spec):
    for op in OPS:
        if op.name == name:
            return op
    shas = {}
    for ver in ("v3", "v4"):
        tmp = DveOpSpec(name=name, opcode=0, uops=lower(spec, ver=ver),
                        rd1_en=_has_src1(spec))
        shas[ver] = tmp.sha(ver)
    op = DveOp(name, spec, subdim=False, uops_sha=shas)
    OPS.append(op)
    CUSTOM_DVE_SPECS[name] = spec
    _dve_ops_mod._SUB_OPCODE_FOR_NAME[name] = _CUSTOM_DVE_ROW_BASE + len(OPS) - 1
    assert _dve_ops_mod._SUB_OPCODE_FOR_NAME[name] < 0x20
    return op


def _uop_h_2x():
    """2X_1PORT program for ANT_LSTM_H: elements lo/hi of min(Src0,1) *
    max(min(Src1,1), C2). Element 0 in blocks 0-3 (result rides delay lane 0
    to the output), element 1 in blocks 4-7 (result = block 7 ALU out).

    Input lanes (lane k feeds block 0's delay chain k-1):
      1:SRC_0->d0  2:ONE->d1  3:SRC_1->d2  4:CONST_2->d3
      5:SRC_0_HI->d4  6:SRC_1_HI->d5
    """
    u = UopConfig()
    for lane, src in ((1, InpSel.SRC_0), (2, InpSel.ONE_F32),
                      (3, InpSel.SRC_1), (4, InpSel.CONST_2),
                      (5, InpSel.SRC_0_HI), (6, InpSel.SRC_1_HI)):
        u.enable_input(src, lane)
    u.require_inp0 = 1
    u.require_inp1 = 1
    u.trigger = (Trigger.SRC_TENSOR_DONE, Trigger.NONE, Trigger.NONE)
    dp = u.datapath_config
    dp[0].enable_alu(UAluOp.MIN, AluInp.PREV_DELAY_2, AluInp.PREV_DELAY_1)
    dp[0].pass_through_delay(0, 1, 3, 4, 5)
    dp[1].enable_alu(UAluOp.MAX, AluInp.PREV_ALU_OUT, AluInp.PREV_DELAY_3)
    dp[1].pass_through_delay(0, 1, 3, 4, 5)
    dp[2].enable_alu(UAluOp.MIN, AluInp.PREV_DELAY_0, AluInp.PREV_DELAY_1)
    dp[2].enable_delay_from_src(DelayInp.PREV_ALU_OUT, 0)   # capture ht_lo
    dp[2].pass_through_delay(1, 3, 4, 5)
    dp[3].enable_alu(UAluOp.MULTIPLY, AluInp.PREV_ALU_OUT, AluInp.PREV_DELAY_0)
    dp[3].pass_through_delay(1, 3, 4, 5)
    dp[4].enable_alu(UAluOp.MIN, AluInp.PREV_DELAY_5, AluInp.PREV_DELAY_1)
    dp[4].enable_delay_from_src(DelayInp.PREV_ALU_OUT, 0)   # capture r_lo
    dp[4].pass_through_delay(1, 3, 4)
    dp[5].enable_alu(UAluOp.MAX, AluInp.PREV_ALU_OUT, AluInp.PREV_DELAY_3)
    dp[5].pass_through_delay(0, 1, 4)
    dp[6].enable_alu(UAluOp.MIN, AluInp.PREV_DELAY_4, AluInp.PREV_DELAY_1)
    dp[6].enable_delay_from_src(DelayInp.PREV_ALU_OUT, 2)   # capture ht_hi
    dp[6].pass_through_delay(0)
    dp[7].enable_alu(UAluOp.MULTIPLY, AluInp.PREV_ALU_OUT, AluInp.PREV_DELAY_2)
    dp[7].pass_through_delay(0)
    u.enable_output(OutSel.DELAY_0, OutPath.WR0_LO)
    u.enable_output(OutSel.ALU_OUT, OutPath.WR0_HI)
    return [u]


def _uop_uc_2x():
    """2X_1PORT program for ANT_LSTM_UC: lo/hi of min(Src0,1) * Src1.
    Lanes: 1:SRC_0->d0 2:ONE->d1 3:SRC_1->d2 4:SRC_0_HI->d3 5:SRC_1_HI->d4."""
    u = UopConfig()
    for lane, src in ((1, InpSel.SRC_0), (2, InpSel.ONE_F32),
                      (3, InpSel.SRC_1), (4, InpSel.SRC_0_HI),
                      (5, InpSel.SRC_1_HI)):
        u.enable_input(src, lane)
    u.require_inp0 = 1
    u.require_inp1 = 1
    u.trigger = (Trigger.SRC_TENSOR_DONE, Trigger.NONE, Trigger.NONE)
    dp = u.datapath_config
    dp[0].enable_alu(UAluOp.MIN, AluInp.PREV_DELAY_0, AluInp.PREV_DELAY_1)
    dp[0].pass_through_delay(1, 2, 3, 4)
    dp[1].enable_alu(UAluOp.MULTIPLY, AluInp.PREV_ALU_OUT, AluInp.PREV_DELAY_2)
    dp[1].pass_through_delay(1, 3, 4)
    dp[2].enable_alu(UAluOp.MIN, AluInp.PREV_DELAY_3, AluInp.PREV_DELAY_1)
    dp[2].enable_delay_from_src(DelayInp.PREV_ALU_OUT, 0)   # capture r_lo
    dp[2].pass_through_delay(4)
    dp[3].enable_alu(UAluOp.MULTIPLY, AluInp.PREV_ALU_OUT, AluInp.PREV_DELAY_4)
    dp[3].pass_through_delay(0)
    for b in range(4, 8):
        dp[b].pass_through_alu()
        dp[b].pass_through_delay(0)
    u.enable_output(OutSel.DELAY_0, OutPath.WR0_LO)
    u.enable_output(OutSel.ALU_OUT, OutPath.WR0_HI)
    return [u]


def _enable_2x(op, uops_2x):
    """Pre-populate the DveOp compile cache with a spec carrying a 2X_1PORT
    uop program so dve_table_for_ops writes the mode entries. Instructions
    opt in per call site via ins.perf_max = 1 (the engine still falls back
    to 1x at runtime when operands are not packed 2-byte)."""
    for ver in ("v3", "v4"):
        spec = DveOpSpec(
            name=op.name,
            opcode=get_dve_sub_opcode(op.name),
            uops=lower(op.spec, ver=ver),
            rd1_en=_has_src1(op.spec),
            uops_2x=uops_2x,
            perf_max=1,
        )
        _COMPILE_CACHE[(op.name, ver)] = spec


def _hs(x, s0, s1):
    return np.minimum(np.maximum(x * s0 + s1, 0.0), 1.0)


def _ht(x, lo):
    return np.maximum(np.minimum(x, 1.0), lo)


# u = min(a_f, 1) * c ; a_f = relu(f') from ACT
OP_UC = _register_op(
    "ANT_LSTM_UC",
    Spec(body=minn(Src0, One) * Src1,
         reference=lambda in0, in1, s0, s1, imm2: np.minimum(in0, 1.0) * in1),
)
# u = hs(f') * c with f' read straight from PSUM (relu+min in-op)
OP_UF = _register_op(
    "ANT_LSTM_UF",
    Spec(body=minn(relu(Src0), One) * Src1,
         reference=lambda in0, in1, s0, s1, imm2:
             np.minimum(np.maximum(in0, 0.0), 1.0) * in1),
)
# h = min(a_o,1)*clip(c, -1, 1); a_o = relu(o') from ACT; imm2=-1.
# Also used for t = min(a_i,1)*ht(g) with in1 = the raw g-gate PSUM bank
# (DVE may read at most one non-scalar operand from PSUM).
OP_H = _register_op(
    "ANT_LSTM_H",
    Spec(body=minn(Src0, One) * maxx(minn(Src1, One), C2),
         reference=lambda in0, in1, s0, s1, imm2:
             np.minimum(in0, 1.0) * np.maximum(np.minimum(in1, 1.0), imm2)),
)
# leaky_relu(psum + bm1) = max(y, 0.01*y), y = Src0 + C1; imm2 = slope
OP_LRELU = _register_op(
    "ANT_LRELU",
    Spec(body=maxx(Src0 + C1, (Src0 + C1) * C2),
         reference=lambda in0, in1, s0, s1, imm2:
             np.maximum(in0 + s1, (in0 + s1) * imm2)),
)

_enable_2x(OP_H, _uop_h_2x())
_enable_2x(OP_UC, _uop_uc_2x())


# ---------------------------------------------------------------- bass build
def build_nc(T, K, BP, use_pack=True, repeat=1, tmp_bufs=4,
             c_add_eng="vector", out_lag=4, h_lag=2, evac_mode="fo_merged",
             pair_mode="ig_fo", out_evac_eng="vector", out_pos="batch2",
             **_unused):
    """Build the per-core Bass program. BP = per-core batch (must be 512).

    PSUM banks (8 x 512 fp32): pa = [i | g] x2 bufs (DVE-read), pb = [f | o]
    x1 buf (ACT-evac'd early), po = out-projection slot banks x2 bufs.
    Biases + hardsigmoid pre-scale folded into the augmented W_ih-[x;1]
    preload on the host: i/f/o psum arrive as (raw+b)/6+0.5, g as raw+b.
    """
    H, MH, C = H_DIM, MH_DIM, C_DIM
    CA = C + 1        # augmented x rows (x0, x1, 1)
    FD = BP           # free dim of every tile
    PSB = 512         # PSUM bank stride in fp32 elems
    RT = K            # rtiles per step
    assert FD == PSB, "layout assumes BP == 512"
    nc = bacc.Bacc("TRN2", target_bir_lowering=False, debug=False)

    phT_e = nc.declare_dram_parameter("phT", [H, K * BP], BF16, isOutput=False)
    xh_e = nc.declare_dram_parameter("xh", [CA, T * BP], BF16, isOutput=False)
    whhT_e = nc.declare_dram_parameter("whhT", [H, 4 * H], BF16, isOutput=False)
    wihT_e = nc.declare_dram_parameter("wihT", [CA, 4 * H], BF16, isOutput=False)
    woutT_e = nc.declare_dram_parameter("woutT", [H, C], BF16, isOutput=False)
    wm1T_e = nc.declare_dram_parameter("wm1T", [H, MH], BF16, isOutput=False)
    wm2T_e = nc.declare_dram_parameter("wm2T", [MH, H], BF16, isOutput=False)
    biasp_e = nc.declare_dram_parameter("biasp", [H, 2], FP32, isOutput=False)
    # out: [128 batch-chunk partitions, T*K*4chunks*2coords] fp32
    NOUT = T * K * 4 * C
    out_e = nc.declare_dram_parameter("out", [128, NOUT], FP32, isOutput=True)

    with tile.TileContext(nc) as tc:
        with ExitStack() as ctx:
            wts = ctx.enter_context(tc.tile_pool(name="wts", bufs=1))
            big = ctx.enter_context(tc.tile_pool(name="big", bufs=1))
            st = ctx.enter_context(tc.tile_pool(name="st", bufs=1))
            tmp = ctx.enter_context(tc.tile_pool(name="tmp", bufs=tmp_bufs))
            psA = ctx.enter_context(tc.tile_pool(name="psA", bufs=2, space="PSUM"))
            psB = ctx.enter_context(tc.tile_pool(name="psB", bufs=2, space="PSUM"))

            # ---- weights / constants into SBUF
            whhT = wts.tile([H, 4 * H], BF16, tag="whhT")
            nc.sync.dma_start(whhT[:], whhT_e[:])
            woutT = wts.tile([H, C], BF16, tag="woutT")
            nc.sync.dma_start(woutT[:], woutT_e[:])
            wm1T = wts.tile([H, MH], BF16, tag="wm1T")
            nc.sync.dma_start(wm1T[:], wm1T_e[:])
            wm2T = wts.tile([MH, H], BF16, tag="wm2T")
            nc.sync.dma_start(wm2T[:], wm2T_e[:])
            biasp = wts.tile([H, 2], FP32, tag="biasp")
            nc.sync.dma_start(biasp[:], biasp_e[:])

            npack = 4 if use_pack else 1
            nprows = 32 * (npack - 1) + CA
            wih = wts.tile([nprows, 4 * H], BF16, tag="wih")
            xrep = wts.tile([nprows, T * BP], BF16, tag="xrep")
            for r in range(npack):
                nc.sync.dma_start(wih[32 * r:32 * r + CA, :], wihT_e[:])
                nc.sync.dma_start(xrep[32 * r:32 * r + CA, :], xh_e[:])

            phT = big.tile([H, K * BP], BF16, tag="phT")
            nc.sync.dma_start(phT[:], phT_e[:])

            # SBUF accumulation buffer for the final output
            out_sb = big.tile([128, NOUT], FP32, tag="out_sb")

            # ---- persistent state tiles
            h_t = [st.tile([H, FD], BF16, name=f"h{j}", tag=f"h{j}")
                   for j in range(RT)]
            c_t = [st.tile([H, FD], BF16, name=f"c{j}", tag=f"c{j}")
                   for j in range(RT)]

            # ---- mapping MLP -> h0
            for j in range(RT):
                pa = psA.tile([H, 2 * PSB], FP32, tag="pa")
                nc.tensor.matmul(pa[0:MH, 0:FD], wm1T[:, 0:MH],
                                 phT[:, j * FD:(j + 1) * FD],
                                 start=True, stop=True)
                a1 = tmp.tile([MH, FD], BF16, tag="a1")
                nc.vector._custom_dve(OP_LRELU, out=a1[:], in0=pa[0:MH, 0:FD],
                                      s1=biasp[0:MH, 0:1], imm2=0.01)
                nc.tensor.matmul(pa[0:H, PSB:PSB + FD], wm2T[:, 0:H], a1[:],
                                 start=True, stop=True)
                nc.scalar.activation(h_t[j][:], pa[0:H, PSB:PSB + FD],
                                     AF.Identity, bias=biasp[:, 1:2], scale=1.0)

            # gate chunk offsets in whhT / wih cols: i=0, f=1, g=2, o=3
            CH = {"i": 0, "f": 1, "g": 2, "o": 3}

            def gcol(name):
                m = CH[name]
                return slice(m * H, (m + 1) * H)

            def gates_mm(ps, xcols, j, names):
                # W_ih-[x;1] preloads (contract=3, row-packed), then W_hh accum
                for r, gname in enumerate(names):
                    rr = (CH[gname] if use_pack else 0)
                    sl = slice(0, FD) if r == 0 else slice(PSB, PSB + FD)
                    nc.tensor.matmul(
                        ps[:, sl],
                        wih[32 * rr:32 * rr + CA, gcol(gname)],
                        xrep[32 * rr:32 * rr + CA, xcols],
                        start=True, stop=False,
                        tile_position=(32 * rr, 0) if use_pack else None,
                    )
                for r, gname in enumerate(names):
                    sl = slice(0, FD) if r == 0 else slice(PSB, PSB + FD)
                    nc.tensor.matmul(ps[:, sl], whhT[:, gcol(gname)],
                                     h_t[j][:], start=False, stop=True)

            # ---- flat software-pipelined main loop over s = tl*RT + j.
            # The [i|g] pa-group runs ONE rtile ahead of the [f|o] pb-group so
            # the PE has ~1.7us of queued work inside the a_fo ACT-evac window
            # (pb is single-buffered; its bank is reusable only after a_fo).
            S = T * repeat * RT
            pa_tiles = {}   # s -> (pa_tile, a_i_tile)
            afo_tiles = {}  # s -> a_fo_tile
            pend_h = []     # s queued for OP_H (lagged h_lag rtiles)
            pend_out = []   # s queued for out-proj (lagged out_lag rtiles)

            def s_tj(s):
                return (s // RT) % T, s % RT

            def emit_pa(s):
                t, j = s_tj(s)
                if pair_mode == "ig_fo":
                    pa = psA.tile([H, 2 * PSB], FP32, tag="pa")  # [i | g]
                    gates_mm(pa, slice(t * BP, (t + 1) * BP), j, "ig")
                    a_i = tmp.tile([H, FD], BF16, tag="ai")
                    nc.scalar.activation(a_i[:], pa[:, 0:FD], AF.Relu,
                                         bias=0.0, scale=1.0)
                    pa_tiles[s] = (pa, a_i[:])
                else:  # gf_io: both halves consumed by DVE straight from psum
                    pa = psA.tile([H, 2 * PSB], FP32, tag="pa")  # [g | f]
                    gates_mm(pa, slice(t * BP, (t + 1) * BP), j, "gf")
                    pa_tiles[s] = (pa, None)

            def emit_h(s):
                t, j = s_tj(s)
                a_o_ap = afo_tiles.pop(s)
                r = nc.vector._custom_dve(OP_H, out=h_t[j][:], in0=a_o_ap,
                                          in1=c_t[j][:], imm2=-1.0)
                r.ins.perf_max = 1   # all-bf16 SBUF -> 2X_1PORT eligible

            def emit_out(s, host_pa, col_off=0, evac=True, evac_cols=8):
                # Transposed out-projection: h 128-batch chunk is stationary,
                # woutT moving -> [128 batch, 2] psum slots. The slots land in
                # the DEAD tail of `host_pa`'s g-bank (already consumed by
                # OP_H-as-t), so no dedicated psum bank is needed and psB can
                # double-buffer. Evac'd to out_sb (8 fp32 per rtile-step).
                t, j = s_tj(s)
                sm = s % (T * RT)
                gend = 2 * PSB if pair_mode == "ig_fo" else PSB
                base = gend - 8 - col_off
                for q in range(4):
                    nc.tensor.matmul(host_pa[:, base + C * q:base + C * (q + 1)],
                                     h_t[j][:, 128 * q:128 * (q + 1)],
                                     woutT[:, 0:C], start=True, stop=True)
                if not evac:
                    return
                elo = base + 8 - evac_cols
                start = (sm + 1) * 8 - evac_cols
                if out_evac_eng == "scalar":
                    nc.scalar.activation(out_sb[:, start:start + evac_cols],
                                         host_pa[:, elo:elo + evac_cols],
                                         AF.Copy, bias=0.0, scale=1.0)
                else:
                    nc.vector.tensor_scalar_add(
                        out_sb[:, start:start + evac_cols],
                        host_pa[:, elo:elo + evac_cols], 0.0)

            emit_pa(0)
            prev_pa = None
            for s in range(S):
                t, j = s_tj(s)
                if out_pos == "early" and prev_pa is not None \
                        and len(pend_out) > out_lag:
                    emit_out(pend_out.pop(0), prev_pa)
                pb = psB.tile([H, 2 * PSB], FP32, tag="pb")
                if pair_mode == "ig_fo":
                    gates_mm(pb, slice(t * BP, (t + 1) * BP), j, "fo")
                    branch = (("o_only" if s % 2 else "fo_merged")
                              if evac_mode == "alt" else evac_mode)
                    if t == 0:
                        branch = "o_only"   # f gate unused when c==0
                    if branch == "o_only":
                        a_o = tmp.tile([H, FD], BF16, tag="ao")
                        nc.scalar.activation(a_o[:], pb[:, PSB:PSB + FD],
                                             AF.Relu, bias=0.0, scale=1.0)
                        afo_tiles[s] = a_o[:]
                        if t > 0:
                            u_d = tmp.tile([H, FD], BF16, tag="uu")
                            nc.vector._custom_dve(OP_UF, out=u_d[:],
                                                  in0=pb[:, 0:FD],
                                                  in1=c_t[j][:])
                    else:
                        a_fo = tmp.tile([H, 2 * PSB], BF16, tag="afo")
                        nc.scalar.activation(a_fo[:], pb[:, 0:2 * PSB],
                                             AF.Relu, bias=0.0, scale=1.0)
                        afo_tiles[s] = a_fo[:, PSB:PSB + FD]
                        if t > 0:
                            u_d = tmp.tile([H, FD], BF16, tag="uu")
                            r = nc.vector._custom_dve(OP_UC, out=u_d[:],
                                                      in0=a_fo[:, 0:FD],
                                                      in1=c_t[j][:])
                            r.ins.perf_max = 1
                else:  # gf_io: pb = [i | o], one merged ACT evac covers both
                    gates_mm(pb, slice(t * BP, (t + 1) * BP), j, "io")
                    a_io = tmp.tile([H, 2 * PSB], BF16, tag="afo")
                    nc.scalar.activation(a_io[:], pb[:, 0:2 * PSB], AF.Relu,
                                         bias=0.0, scale=1.0)
                    afo_tiles[s] = a_io[:, PSB:PSB + FD]   # a_o
                if s + 1 < S:
                    emit_pa(s + 1)   # prefetched pa pair for the next rtile

                # DVE: t = min(a_i,1)*ht(g); g raw+b read from psum.
                # The g-bank tail (last 8 cols) doubles as the out-proj slot
                # region after this read (WAR-sem'd by the tile pool).
                pa, a_i = pa_tiles.pop(s)
                if pair_mode == "gf_io":
                    a_i = a_io[:, 0:FD]
                    g_ap = pa[:, 0:FD]
                    if t > 0:
                        u_d = tmp.tile([H, FD], BF16, tag="uu")
                        nc.vector._custom_dve(OP_UF, out=u_d[:],
                                              in0=pa[:, PSB:PSB + FD],
                                              in1=c_t[j][:])
                else:
                    g_ap = pa[:, PSB:PSB + FD]
                if t == 0:
                    t_dst = c_t[j]      # c0 = 0 -> c1 = t
                else:
                    t_dst = tmp.tile([H, FD], BF16, tag="tt")
                nc.vector._custom_dve(OP_H, out=t_dst[:], in0=a_i,
                                      in1=g_ap, imm2=-1.0)
                # lagged OP_H (waits out the GPSIMD add latency off-stream)
                if len(pend_h) >= h_lag:
                    emit_h(pend_h.pop(0))
                if t > 0:
                    add_e = getattr(nc, c_add_eng)
                    add_e.tensor_add(c_t[j][:], u_d[:], t_dst[:])
                pend_h.append(s)
                if j == RT - 1:      # step boundary: drain so h is final
                    while pend_h:
                        emit_h(pend_h.pop(0))

                pend_out.append(s)
                if out_pos == "late" and len(pend_out) > out_lag:
                    emit_out(pend_out.pop(0), pa)
                elif out_pos == "batch2" and len(pend_out) > out_lag + 1 \
                        and s % 2 == 1:
                    # two lagged out-projs share one host tile + one evac
                    s0, s1 = pend_out[0], pend_out[1]
                    if s0 % (T * RT) + 1 == s1 % (T * RT):
                        emit_out(pend_out.pop(0), pa, col_off=8, evac=False)
                        emit_out(pend_out.pop(0), pa, col_off=0, evac=True,
                                 evac_cols=16)
                    else:   # repeat-boundary wrap: evac singly
                        emit_out(pend_out.pop(0), pa, col_off=8)
                        emit_out(pend_out.pop(0), pa, col_off=0)
                prev_pa = pa
            last_pa = pa
            for k in range(len(pend_out)):
                emit_out(pend_out[k], last_pa, col_off=8 * (k + 1))
            pend_out.clear()

            nc.sync.dma_start(out_e[:], out_sb[:])

    nc.finalize()
    return nc


# ---------------------------------------------------------------- host side
def _bf16(x):
    return np.ascontiguousarray(x, dtype=np.float32).astype(ml_dtypes.bfloat16)


def prep_core_inputs(inputs, core, T, K, BP):
    H, MH, C = H_DIM, MH_DIM, C_DIM
    b0 = core * BP
    ph = np.asarray(inputs["pred_lstm_hidden"], np.float32)[:, b0:b0 + BP, :]
    phT = ph.transpose(2, 0, 1).reshape(H, K * BP)
    idx = np.concatenate([[0], np.arange(T - 1)])
    obs = np.asarray(inputs["obs_traj_rel"], np.float32)
    xs = obs[idx][:, b0:b0 + BP, :C]
    xh = xs.transpose(2, 0, 1).reshape(C, T * BP)
    xh = np.concatenate([xh, np.ones((1, T * BP), np.float32)], axis=0)
    bsum = (np.asarray(inputs["b_ih"], np.float32)
            + np.asarray(inputs["b_hh"], np.float32))
    # per-gate-chunk scale and bias folded into W_hh / W_ih / the x=1 row:
    #   i, f, o chunks: psum = (raw + b)/6 + 0.5 ; g chunk: psum = raw + b
    scale = np.ones(4 * H, np.float32) / 6.0
    scale[2 * H:3 * H] = 1.0
    bias_row = bsum * scale
    bias_row[0:2 * H] += 0.5
    bias_row[3 * H:4 * H] += 0.5
    whh_s = np.asarray(inputs["W_hh"], np.float32) * scale[:, None]
    wih_s = np.asarray(inputs["W_ih"], np.float32) * scale[:, None]
    wih_aug = np.concatenate([wih_s.T, bias_row[None, :]], axis=0)  # [3, 4H]
    biasp = np.zeros((H, 2), np.float32)
    biasp[0:MH, 0] = np.asarray(inputs["bm1"], np.float32)
    biasp[:, 1] = np.asarray(inputs["bm2"], np.float32)
    return {
        "phT": _bf16(phT),
        "xh": _bf16(xh),
        "whhT": _bf16(whh_s.T),
        "wihT": _bf16(wih_aug),
        "woutT": _bf16(np.asarray(inputs["W_out"], np.float32).T),
        "wm1T": _bf16(np.asarray(inputs["Wm1"], np.float32).T),
        "wm2T": _bf16(np.asarray(inputs["Wm2"], np.float32).T),
        "biasp": biasp,
    }


_NC_CACHE = {}


def _get_nc(T, K, BP):
    key = (T, K, BP)
    if key not in _NC_CACHE:
        _NC_CACHE[key] = build_nc(T, K, BP)
    return _NC_CACHE[key]


def kernel(**inputs) -> np.ndarray:
    T, K, B, C = T_FULL, K_FULL, B_FULL, C_DIM
    BP = B // N_CORES
    nc = _get_nc(T, K, BP)
    in_maps = [prep_core_inputs(inputs, c, T, K, BP) for c in range(N_CORES)]
    trace = bool(int(os.environ.get("KERNEL_TRACE", "0")))
    res = run_bass_kernel_spmd(nc, in_maps, list(range(N_CORES)), trace=trace)
    if trace:
        kernel.last_exec_time_ns = res.exec_time_ns
        kernel.last_results = res
    # per-core out: [128, T*K*4*C] -> [T, K, BP, C]
    parts = []
    for c in range(N_CORES):
        arr = res.results[c]["out"].reshape(128, T, K, 4, C)
        parts.append(arr.transpose(1, 2, 3, 0, 4).reshape(T, K, BP, C))
    full = np.concatenate(parts, axis=2)  # [T, K, B, C]
    b_out = np.asarray(inputs["b_out"], np.float32)
    return np.ascontiguousarray(full + b_out, dtype=np.float32)


# revision 48
# speedup vs baseline: 1.7626x; 1.0271x over previous
"""Trainium2 Bass kernel for nn_Decoder (mapping MLP + hard-LSTM scan + out proj).

Self-contained: takes FULL inputs (as produced by setup_inputs), shards batch
across 8 NeuronCores, runs a Bass/Tile kernel via run_bass_kernel_spmd, and
gathers the full [T, K, B, C] output.

Layout per core (B' = B/8 = 512 batch elems):
  rows = k*B' + b  (20 "rtiles" of 512 rows each, one per k)
  h, c state: [H=128 partitions, 512 rows] bf16 tiles, one pair per rtile.

Per step & rtile (engine assignment tuned against the TRN2 cost model):
  PE   : 4 W_ih-[x;1] preloads + 4 W_hh matmuls fill two PSUM bank-pairs
         [i|g] and [f|o]; the out-projection runs TRANSPOSED (h 128-batch
         chunk as stationary, W_outT as moving) so it costs ~2 columns
         instead of 512, accumulating [128 batch, 2] slots in a PSUM bank.
  ACT  : one merged relu evac of [f|o] -> a_fo bf16; PSUM out-bank evac
         every 64 rtile-steps.
  DVE  : OP_T3 t = hs(i)*ht(g) straight from both PSUM banks;
         OP_UC u = min(a_f,1)*c; OP_H h = min(a_o,1)*ht(c) (lagged 1 rtile
         so the GPSIMD add can complete without stalling the DVE stream).
  POOL : c = u + t (native tensor add on the otherwise idle GPSIMD engine).
"""
import os
import sys

sys.path.insert(0, "/opt/trn_rl_repo")

import numpy as np
import ml_dtypes
from contextlib import ExitStack

import concourse.bass as bass
import concourse.tile as tile
from concourse import mybir, bacc
import concourse.dve_ops as _dve_ops_mod
from concourse.dve_ops import (
    DveOp, OPS, CUSTOM_DVE_SPECS, _CUSTOM_DVE_ROW_BASE, get_dve_sub_opcode,
    _COMPILE_CACHE,
)
from concourse.dve_spec import (
    Spec, Src0, Src1, C0, C1, C2, Zero, One, maxx, minn, relu, lower, _has_src1,
)
from concourse.dve_uop import (
    DveOpSpec, UopConfig, UopDpConfig, InpSel, OutSel, OutPath, AluInp,
    AluOp as UAluOp, DelayInp, Trigger,
)
from concourse.bass_utils import run_bass_kernel_spmd

FP32 = mybir.dt.float32
BF16 = mybir.dt.bfloat16
AF = mybir.ActivationFunctionType

# Full-problem config (hardcoded; the harness always calls with these shapes).
T_FULL, K_FULL, B_FULL, C_DIM, H_DIM, MH_DIM, N_CORES = 20, 20, 4096, 2, 128, 64, 8


# ---------------------------------------------------------------- custom ops
def _register_op(name, spec):
    for op in OPS:
        if op.name == name:
            return op
    shas = {}
    for ver in ("v3", "v4"):
        tmp = DveOpSpec(name=name, opcode=0, uops=lower(spec, ver=ver),
                        rd1_en=_has_src1(spec))
        shas[ver] = tmp.sha(ver)
    op = DveOp(name, spec, subdim=False, uops_sha=shas)
    OPS.append(op)
    CUSTOM_DVE_SPECS[name] = spec
    _dve_ops_mod._SUB_OPCODE_FOR_NAME[name] = _CUSTOM_DVE_ROW_BASE + len(OPS) - 1
    assert _dve_ops_mod._SUB_OPCODE_FOR_NAME[name] < 0x20
    return op


def _uop_h_2x():
    """2X_1PORT program for ANT_LSTM_H: elements lo/hi of min(Src0,1) *
    max(min(Src1,1), C2). Element 0 in blocks 0-3 (result rides delay lane 0
    to the output), element 1 in blocks 4-7 (result = block 7 ALU out).

    Input lanes (lane k feeds block 0's delay chain k-1):
      1:SRC_0->d0  2:ONE->d1  3:SRC_1->d2  4:CONST_2->d3
      5:SRC_0_HI->d4  6:SRC_1_HI->d5
    """
    u = UopConfig()
    for lane, src in ((1, InpSel.SRC_0), (2, InpSel.ONE_F32),
                      (3, InpSel.SRC_1), (4, InpSel.CONST_2),
                      (5, InpSel.SRC_0_HI), (6, InpSel.SRC_1_HI)):
        u.enable_input(src, lane)
    u.require_inp0 = 1
    u.require_inp1 = 1
    u.trigger = (Trigger.SRC_TENSOR_DONE, Trigger.NONE, Trigger.NONE)
    dp = u.datapath_config
    dp[0].enable_alu(UAluOp.MIN, AluInp.PREV_DELAY_2, AluInp.PREV_DELAY_1)
    dp[0].pass_through_delay(0, 1, 3, 4, 5)
    dp[1].enable_alu(UAluOp.MAX, AluInp.PREV_ALU_OUT, AluInp.PREV_DELAY_3)
    dp[1].pass_through_delay(0, 1, 3, 4, 5)
    dp[2].enable_alu(UAluOp.MIN, AluInp.PREV_DELAY_0, AluInp.PREV_DELAY_1)
    dp[2].enable_delay_from_src(DelayInp.PREV_ALU_OUT, 0)   # capture ht_lo
    dp[2].pass_through_delay(1, 3, 4, 5)
    dp[3].enable_alu(UAluOp.MULTIPLY, AluInp.PREV_ALU_OUT, AluInp.PREV_DELAY_0)
    dp[3].pass_through_delay(1, 3, 4, 5)
    dp[4].enable_alu(UAluOp.MIN, AluInp.PREV_DELAY_5, AluInp.PREV_DELAY_1)
    dp[4].enable_delay_from_src(DelayInp.PREV_ALU_OUT, 0)   # capture r_lo
    dp[4].pass_through_delay(1, 3, 4)
    dp[5].enable_alu(UAluOp.MAX, AluInp.PREV_ALU_OUT, AluInp.PREV_DELAY_3)
    dp[5].pass_through_delay(0, 1, 4)
    dp[6].enable_alu(UAluOp.MIN, AluInp.PREV_DELAY_4, AluInp.PREV_DELAY_1)
    dp[6].enable_delay_from_src(DelayInp.PREV_ALU_OUT, 2)   # capture ht_hi
    dp[6].pass_through_delay(0)
    dp[7].enable_alu(UAluOp.MULTIPLY, AluInp.PREV_ALU_OUT, AluInp.PREV_DELAY_2)
    dp[7].pass_through_delay(0)
    u.enable_output(OutSel.DELAY_0, OutPath.WR0_LO)
    u.enable_output(OutSel.ALU_OUT, OutPath.WR0_HI)
    return [u]


def _uop_uc_2x():
    """2X_1PORT program for ANT_LSTM_UC: lo/hi of min(Src0,1) * Src1.
    Lanes: 1:SRC_0->d0 2:ONE->d1 3:SRC_1->d2 4:SRC_0_HI->d3 5:SRC_1_HI->d4."""
    u = UopConfig()
    for lane, src in ((1, InpSel.SRC_0), (2, InpSel.ONE_F32),
                      (3, InpSel.SRC_1), (4, InpSel.SRC_0_HI),
                      (5, InpSel.SRC_1_HI)):
        u.enable_input(src, lane)
    u.require_inp0 = 1
    u.require_inp1 = 1
    u.trigger = (Trigger.SRC_TENSOR_DONE, Trigger.NONE, Trigger.NONE)
    dp = u.datapath_config
    dp[0].enable_alu(UAluOp.MIN, AluInp.PREV_DELAY_0, AluInp.PREV_DELAY_1)
    dp[0].pass_through_delay(1, 2, 3, 4)
    dp[1].enable_alu(UAluOp.MULTIPLY, AluInp.PREV_ALU_OUT, AluInp.PREV_DELAY_2)
    dp[1].pass_through_delay(1, 3, 4)
    dp[2].enable_alu(UAluOp.MIN, AluInp.PREV_DELAY_3, AluInp.PREV_DELAY_1)
    dp[2].enable_delay_from_src(DelayInp.PREV_ALU_OUT, 0)   # capture r_lo
    dp[2].pass_through_delay(4)
    dp[3].enable_alu(UAluOp.MULTIPLY, AluInp.PREV_ALU_OUT, AluInp.PREV_DELAY_4)
    dp[3].pass_through_delay(0)
    for b in range(4, 8):
        dp[b].pass_through_alu()
        dp[b].pass_through_delay(0)
    u.enable_output(OutSel.DELAY_0, OutPath.WR0_LO)
    u.enable_output(OutSel.ALU_OUT, OutPath.WR0_HI)
    return [u]


def _enable_2x(op, uops_2x):
    """Pre-populate the DveOp compile cache with a spec carrying a 2X_1PORT
    uop program so dve_table_for_ops writes the mode entries. Instructions
    opt in per call site via ins.perf_max = 1 (the engine still falls back
    to 1x at runtime when operands are not packed 2-byte)."""
    for ver in ("v3", "v4"):
        spec = DveOpSpec(
            name=op.name,
            opcode=get_dve_sub_opcode(op.name),
            uops=lower(op.spec, ver=ver),
            rd1_en=_has_src1(op.spec),
            uops_2x=uops_2x,
            perf_max=1,
        )
        _COMPILE_CACHE[(op.name, ver)] = spec


def _hs(x, s0, s1):
    return np.minimum(np.maximum(x * s0 + s1, 0.0), 1.0)


def _ht(x, lo):
    return np.maximum(np.minimum(x, 1.0), lo)


# u = min(a_f, 1) * c ; a_f = relu(f') from ACT
OP_UC = _register_op(
    "ANT_LSTM_UC",
    Spec(body=minn(Src0, One) * Src1,
         reference=lambda in0, in1, s0, s1, imm2: np.minimum(in0, 1.0) * in1),
)
# u = hs(f') * c with f' read straight from PSUM (relu+min in-op)
OP_UF = _register_op(
    "ANT_LSTM_UF",
    Spec(body=minn(relu(Src0), One) * Src1,
         reference=lambda in0, in1, s0, s1, imm2:
             np.minimum(np.maximum(in0, 0.0), 1.0) * in1),
)
# h = min(a_o,1)*clip(c, -1, 1); a_o = relu(o') from ACT; imm2=-1.
# Also used for t = min(a_i,1)*ht(g) with in1 = the raw g-gate PSUM bank
# (DVE may read at most one non-scalar operand from PSUM).
OP_H = _register_op(
    "ANT_LSTM_H",
    Spec(body=minn(Src0, One) * maxx(minn(Src1, One), C2),
         reference=lambda in0, in1, s0, s1, imm2:
             np.minimum(in0, 1.0) * np.maximum(np.minimum(in1, 1.0), imm2)),
)
# leaky_relu(psum + bm1) = max(y, 0.01*y), y = Src0 + C1; imm2 = slope
OP_LRELU = _register_op(
    "ANT_LRELU",
    Spec(body=maxx(Src0 + C1, (Src0 + C1) * C2),
         reference=lambda in0, in1, s0, s1, imm2:
             np.maximum(in0 + s1, (in0 + s1) * imm2)),
)

_enable_2x(OP_H, _uop_h_2x())
_enable_2x(OP_UC, _uop_uc_2x())


# ---------------------------------------------------------------- bass build
def build_nc(T, K, BP, use_pack=False, repeat=1, tmp_bufs=5,
             c_add_eng="vector", out_lag=2, h_lag=2, evac_mode="fo_merged",
             pair_mode="ig_fo", out_evac_eng="vector", out_pos="batch2",
             **_unused):
    """Build the per-core Bass program. BP = per-core batch (must be 512).

    PSUM banks (8 x 512 fp32): pa = [i | g] x2 bufs (DVE-read), pb = [f | o]
    x1 buf (ACT-evac'd early), po = out-projection slot banks x2 bufs.
    Biases + hardsigmoid pre-scale folded into the augmented W_ih-[x;1]
    preload on the host: i/f/o psum arrive as (raw+b)/6+0.5, g as raw+b.
    """
    H, MH, C = H_DIM, MH_DIM, C_DIM
    CA = C + 1        # augmented x rows (x0, x1, 1)
    FD = BP           # free dim of every tile
    PSB = 512         # PSUM bank stride in fp32 elems
    RT = K            # rtiles per step
    assert FD == PSB, "layout assumes BP == 512"
    nc = bacc.Bacc("TRN2", target_bir_lowering=False, debug=False)

    phT_e = nc.declare_dram_parameter("phT", [H, K * BP], BF16, isOutput=False)
    xh_e = nc.declare_dram_parameter("xh", [CA, T * BP], BF16, isOutput=False)
    whhT_e = nc.declare_dram_parameter("whhT", [H, 4 * H], BF16, isOutput=False)
    wihT_e = nc.declare_dram_parameter("wihT", [CA, 4 * H], BF16, isOutput=False)
    woutT_e = nc.declare_dram_parameter("woutT", [H, C], BF16, isOutput=False)
    wm1T_e = nc.declare_dram_parameter("wm1T", [H, MH], BF16, isOutput=False)
    wm2T_e = nc.declare_dram_parameter("wm2T", [MH, H], BF16, isOutput=False)
    biasp_e = nc.declare_dram_parameter("biasp", [H, 2], FP32, isOutput=False)
    # out: [128 batch-chunk partitions, T*K*4chunks*2coords] fp32
    NOUT = T * K * 4 * C
    out_e = nc.declare_dram_parameter("out", [128, NOUT], FP32, isOutput=True)

    with tile.TileContext(nc) as tc:
        with ExitStack() as ctx:
            wts = ctx.enter_context(tc.tile_pool(name="wts", bufs=1))
            big = ctx.enter_context(tc.tile_pool(name="big", bufs=1))
            st = ctx.enter_context(tc.tile_pool(name="st", bufs=1))
            tmp = ctx.enter_context(tc.tile_pool(name="tmp", bufs=tmp_bufs))
            psA = ctx.enter_context(tc.tile_pool(name="psA", bufs=2, space="PSUM"))
            psB = ctx.enter_context(tc.tile_pool(name="psB", bufs=2, space="PSUM"))

            # ---- weights / constants into SBUF
            whhT = wts.tile([H, 4 * H], BF16, tag="whhT")
            nc.sync.dma_start(whhT[:], whhT_e[:])
            woutT = wts.tile([H, C], BF16, tag="woutT")
            nc.sync.dma_start(woutT[:], woutT_e[:])
            wm1T = wts.tile([H, MH], BF16, tag="wm1T")
            nc.sync.dma_start(wm1T[:], wm1T_e[:])
            wm2T = wts.tile([MH, H], BF16, tag="wm2T")
            nc.sync.dma_start(wm2T[:], wm2T_e[:])
            biasp = wts.tile([H, 2], FP32, tag="biasp")
            nc.sync.dma_start(biasp[:], biasp_e[:])

            npack = 4 if use_pack else 1
            nprows = 32 * (npack - 1) + CA
            wih = wts.tile([nprows, 4 * H], BF16, tag="wih")
            xrep = wts.tile([nprows, T * BP], BF16, tag="xrep")
            phT = big.tile([H, K * BP], BF16, tag="phT")
            HK = K * BP // 2
            # first halves first: MLP(0) / step-0 gates wait ~half the DMA
            nc.sync.dma_start(phT[:, 0:HK], phT_e[:, 0:HK])
            for r in range(npack):
                nc.sync.dma_start(wih[32 * r:32 * r + CA, :], wihT_e[:])
                nc.sync.dma_start(xrep[32 * r:32 * r + CA, 0:BP],
                                  xh_e[:, 0:BP])
                nc.sync.dma_start(xrep[32 * r:32 * r + CA, BP:T * BP],
                                  xh_e[:, BP:T * BP])
            nc.sync.dma_start(phT[:, HK:], phT_e[:, HK:])

            # SBUF accumulation buffer for the final output
            out_sb = big.tile([128, NOUT], FP32, tag="out_sb")

            # ---- persistent state tiles
            h_t = [st.tile([H, FD], BF16, name=f"h{j}", tag=f"h{j}")
                   for j in range(RT)]
            c_t = [st.tile([H, FD], BF16, name=f"c{j}", tag=f"c{j}")
                   for j in range(RT)]

            # ---- mapping MLP -> h0 (interleaved into the first step's
            # pipeline: MLP(j) shares the psA pool rotation and is emitted
            # two rtiles ahead of the gate matmuls that consume h_t[j])
            def emit_mlp(j):
                pm = psA.tile([H, 2 * PSB], FP32, tag="pa", name="pm")
                nc.tensor.matmul(pm[0:MH, 0:FD], wm1T[:, 0:MH],
                                 phT[:, j * FD:(j + 1) * FD],
                                 start=True, stop=True)
                a1 = tmp.tile([MH, FD], BF16, tag="a1")
                nc.vector._custom_dve(OP_LRELU, out=a1[:], in0=pm[0:MH, 0:FD],
                                      s1=biasp[0:MH, 0:1], imm2=0.01)
                nc.tensor.matmul(pm[0:H, PSB:PSB + FD], wm2T[:, 0:H], a1[:],
                                 start=True, stop=True)
                nc.scalar.activation(h_t[j][:], pm[0:H, PSB:PSB + FD],
                                     AF.Identity, bias=biasp[:, 1:2], scale=1.0)

            for j in range(RT):
                emit_mlp(j)

            # gate chunk offsets in whhT / wih cols: i=0, f=1, g=2, o=3
            CH = {"i": 0, "f": 1, "g": 2, "o": 3}

            def gcol(name):
                m = CH[name]
                return slice(m * H, (m + 1) * H)

            def gates_mm(ps, xcols, j, names):
                # W_ih-[x;1] preloads (contract=3, row-packed), then W_hh accum
                for r, gname in enumerate(names):
                    rr = (CH[gname] if use_pack else 0)
                    sl = slice(0, FD) if r == 0 else slice(PSB, PSB + FD)
                    nc.tensor.matmul(
                        ps[:, sl],
                        wih[32 * rr:32 * rr + CA, gcol(gname)],
                        xrep[32 * rr:32 * rr + CA, xcols],
                        start=True, stop=False,
                        tile_position=(32 * rr, 0) if use_pack else None,
                    )
                for r, gname in enumerate(names):
                    sl = slice(0, FD) if r == 0 else slice(PSB, PSB + FD)
                    nc.tensor.matmul(ps[:, sl], whhT[:, gcol(gname)],
                                     h_t[j][:], start=False, stop=True)

            # ---- flat software-pipelined main loop over s = tl*RT + j.
            # The [i|g] pa-group runs ONE rtile ahead of the [f|o] pb-group so
            # the PE has ~1.7us of queued work inside the a_fo ACT-evac window
            # (pb is single-buffered; its bank is reusable only after a_fo).
            S = T * repeat * RT
            pa_tiles = {}   # s -> (pa_tile, a_i_tile)
            afo_tiles = {}  # s -> a_fo_tile
            pend_h = []     # s queued for OP_H (lagged h_lag rtiles)
            pend_out = []   # s queued for out-proj (lagged out_lag rtiles)

            def s_tj(s):
                return (s // RT) % T, s % RT

            def emit_pa(s):
                t, j = s_tj(s)
                if pair_mode == "ig_fo":
                    pa = psA.tile([H, 2 * PSB], FP32, tag="pa")  # [i | g]
                    gates_mm(pa, slice(t * BP, (t + 1) * BP), j, "ig")
                    a_i = tmp.tile([H, FD], BF16, tag="ai")
                    nc.scalar.activation(a_i[:], pa[:, 0:FD], AF.Relu,
                                         bias=0.0, scale=1.0)
                    pa_tiles[s] = (pa, a_i[:])
                else:  # gf_io: both halves consumed by DVE straight from psum
                    pa = psA.tile([H, 2 * PSB], FP32, tag="pa")  # [g | f]
                    gates_mm(pa, slice(t * BP, (t + 1) * BP), j, "gf")
                    pa_tiles[s] = (pa, None)

            def emit_h(s):
                t, j = s_tj(s)
                a_o_ap = afo_tiles.pop(s)
                r = nc.vector._custom_dve(OP_H, out=h_t[j][:], in0=a_o_ap,
                                          in1=c_t[j][:], imm2=-1.0)
                r.ins.perf_max = 1   # all-bf16 SBUF -> 2X_1PORT eligible

            def emit_out(s, host_pa, col_off=0, evac=True, evac_cols=8):
                # Transposed out-projection: h 128-batch chunk is stationary,
                # woutT moving -> [128 batch, 2] psum slots. The slots land in
                # the DEAD tail of `host_pa`'s g-bank (already consumed by
                # OP_H-as-t), so no dedicated psum bank is needed and psB can
                # double-buffer. Evac'd to out_sb (8 fp32 per rtile-step).
                t, j = s_tj(s)
                sm = s % (T * RT)
                gend = 2 * PSB if pair_mode == "ig_fo" else PSB
                base = gend - 8 - col_off
                for q in range(4):
                    nc.tensor.matmul(host_pa[:, base + C * q:base + C * (q + 1)],
                                     h_t[j][:, 128 * q:128 * (q + 1)],
                                     woutT[:, 0:C], start=True, stop=True)
                if not evac:
                    return
                elo = base + 8 - evac_cols
                start = (sm + 1) * 8 - evac_cols
                if out_evac_eng == "scalar":
                    nc.scalar.activation(out_sb[:, start:start + evac_cols],
                                         host_pa[:, elo:elo + evac_cols],
                                         AF.Copy, bias=0.0, scale=1.0)
                else:
                    nc.vector.tensor_scalar_add(
                        out_sb[:, start:start + evac_cols],
                        host_pa[:, elo:elo + evac_cols], 0.0)
                # stream completed 512-col out_sb regions to DRAM during the
                # loop so no big DMA sits in the drain tail (last repeat only)
                if s >= S - T * RT:
                    end = start + evac_cols
                    if end % 512 == 0 or end == NOUT:
                        lo = (end - 1) // 512 * 512
                        nc.sync.dma_start(out_e[:, lo:end], out_sb[:, lo:end])

            emit_pa(0)
            prev_pa = None
            for s in range(S):
                t, j = s_tj(s)
                if out_pos == "early" and prev_pa is not None \
                        and len(pend_out) > out_lag:
                    emit_out(pend_out.pop(0), prev_pa)
                pb = psB.tile([H, 2 * PSB], FP32, tag="pb")
                if pair_mode == "ig_fo":
                    gates_mm(pb, slice(t * BP, (t + 1) * BP), j, "fo")
                    branch = (("o_only" if s % 2 else "fo_merged")
                              if evac_mode == "alt" else evac_mode)
                    if t == 0:
                        branch = "o_only"   # f gate unused when c==0
                    if branch == "o_only":
                        a_o = tmp.tile([H, FD], BF16, tag="ao")
                        nc.scalar.activation(a_o[:], pb[:, PSB:PSB + FD],
                                             AF.Relu, bias=0.0, scale=1.0)
                        afo_tiles[s] = a_o[:]
                        if t > 0:
                            u_d = tmp.tile([H, FD], BF16, tag="uu")
                            nc.vector._custom_dve(OP_UF, out=u_d[:],
                                                  in0=pb[:, 0:FD],
                                                  in1=c_t[j][:])
                    else:
                        a_fo = tmp.tile([H, 2 * PSB], BF16, tag="afo")
                        nc.scalar.activation(a_fo[:], pb[:, 0:2 * PSB],
                                             AF.Relu, bias=0.0, scale=1.0)
                        afo_tiles[s] = a_fo[:, PSB:PSB + FD]
                        if t > 0:
                            u_d = tmp.tile([H, FD], BF16, tag="uu")
                            r = nc.vector._custom_dve(OP_UC, out=u_d[:],
                                                      in0=a_fo[:, 0:FD],
                                                      in1=c_t[j][:])
                            r.ins.perf_max = 1
                else:  # gf_io: pb = [i | o], one merged ACT evac covers both
                    gates_mm(pb, slice(t * BP, (t + 1) * BP), j, "io")
                    a_io = tmp.tile([H, 2 * PSB], BF16, tag="afo")
                    nc.scalar.activation(a_io[:], pb[:, 0:2 * PSB], AF.Relu,
                                         bias=0.0, scale=1.0)
                    afo_tiles[s] = a_io[:, PSB:PSB + FD]   # a_o
                if s + 1 < S:
                    emit_pa(s + 1)   # prefetched pa pair for the next rtile

                # DVE: t = min(a_i,1)*ht(g); g raw+b read from psum.
                # The g-bank tail (last 8 cols) doubles as the out-proj slot
                # region after this read (WAR-sem'd by the tile pool).
                pa, a_i = pa_tiles.pop(s)
                if pair_mode == "gf_io":
                    a_i = a_io[:, 0:FD]
                    g_ap = pa[:, 0:FD]
                    if t > 0:
                        u_d = tmp.tile([H, FD], BF16, tag="uu")
                        nc.vector._custom_dve(OP_UF, out=u_d[:],
                                              in0=pa[:, PSB:PSB + FD],
                                              in1=c_t[j][:])
                else:
                    g_ap = pa[:, PSB:PSB + FD]
                if t == 0:
                    t_dst = c_t[j]      # c0 = 0 -> c1 = t
                else:
                    t_dst = tmp.tile([H, FD], BF16, tag="tt")
                nc.vector._custom_dve(OP_H, out=t_dst[:], in0=a_i,
                                      in1=g_ap, imm2=-1.0)
                # lagged OP_H (waits out the GPSIMD add latency off-stream)
                if len(pend_h) >= h_lag:
                    emit_h(pend_h.pop(0))
                if t > 0:
                    add_e = getattr(nc, c_add_eng)
                    add_e.tensor_add(c_t[j][:], u_d[:], t_dst[:])
                pend_h.append(s)
                if j == RT - 1:      # step boundary: drain so h is final
                    while pend_h:
                        emit_h(pend_h.pop(0))

                pend_out.append(s)
                if out_pos == "late" and len(pend_out) > out_lag:
                    emit_out(pend_out.pop(0), pa)
                elif out_pos == "batch2" and len(pend_out) > out_lag + 1 \
                        and s % 2 == 1:
                    # two lagged out-projs share one host tile + one evac
                    s0, s1 = pend_out[0], pend_out[1]
                    if s0 % (T * RT) + 1 == s1 % (T * RT):
                        emit_out(pend_out.pop(0), pa, col_off=8, evac=False)
                        emit_out(pend_out.pop(0), pa, col_off=0, evac=True,
                                 evac_cols=16)
                    else:   # repeat-boundary wrap: evac singly
                        emit_out(pend_out.pop(0), pa, col_off=8)
                        emit_out(pend_out.pop(0), pa, col_off=0)
                prev_pa = pa
            last_pa = pa
            for k in range(len(pend_out)):
                emit_out(pend_out[k], last_pa, col_off=8 * (k + 1))
            pend_out.clear()

    nc.finalize()
    return nc


# ---------------------------------------------------------------- host side
def _bf16(x):
    return np.ascontiguousarray(x, dtype=np.float32).astype(ml_dtypes.bfloat16)


def prep_core_inputs(inputs, core, T, K, BP):
    H, MH, C = H_DIM, MH_DIM, C_DIM
    b0 = core * BP
    ph = np.asarray(inputs["pred_lstm_hidden"], np.float32)[:, b0:b0 + BP, :]
    phT = ph.transpose(2, 0, 1).reshape(H, K * BP)
    idx = np.concatenate([[0], np.arange(T - 1)])
    obs = np.asarray(inputs["obs_traj_rel"], np.float32)
    xs = obs[idx][:, b0:b0 + BP, :C]
    xh = xs.transpose(2, 0, 1).reshape(C, T * BP)
    xh = np.concatenate([xh, np.ones((1, T * BP), np.float32)], axis=0)
    bsum = (np.asarray(inputs["b_ih"], np.float32)
            + np.asarray(inputs["b_hh"], np.float32))
    # per-gate-chunk scale and bias folded into W_hh / W_ih / the x=1 row:
    #   i, f, o chunks: psum = (raw + b)/6 + 0.5 ; g chunk: psum = raw + b
    scale = np.ones(4 * H, np.float32) / 6.0
    scale[2 * H:3 * H] = 1.0
    bias_row = bsum * scale
    bias_row[0:2 * H] += 0.5
    bias_row[3 * H:4 * H] += 0.5
    whh_s = np.asarray(inputs["W_hh"], np.float32) * scale[:, None]
    wih_s = np.asarray(inputs["W_ih"], np.float32) * scale[:, None]
    wih_aug = np.concatenate([wih_s.T, bias_row[None, :]], axis=0)  # [3, 4H]
    biasp = np.zeros((H, 2), np.float32)
    biasp[0:MH, 0] = np.asarray(inputs["bm1"], np.float32)
    biasp[:, 1] = np.asarray(inputs["bm2"], np.float32)
    return {
        "phT": _bf16(phT),
        "xh": _bf16(xh),
        "whhT": _bf16(whh_s.T),
        "wihT": _bf16(wih_aug),
        "woutT": _bf16(np.asarray(inputs["W_out"], np.float32).T),
        "wm1T": _bf16(np.asarray(inputs["Wm1"], np.float32).T),
        "wm2T": _bf16(np.asarray(inputs["Wm2"], np.float32).T),
        "biasp": biasp,
    }


_NC_CACHE = {}


def _get_nc(T, K, BP):
    key = (T, K, BP)
    if key not in _NC_CACHE:
        _NC_CACHE[key] = build_nc(T, K, BP)
    return _NC_CACHE[key]


def kernel(**inputs) -> np.ndarray:
    T, K, B, C = T_FULL, K_FULL, B_FULL, C_DIM
    BP = B // N_CORES
    nc = _get_nc(T, K, BP)
    in_maps = [prep_core_inputs(inputs, c, T, K, BP) for c in range(N_CORES)]
    trace = bool(int(os.environ.get("KERNEL_TRACE", "0")))
    res = run_bass_kernel_spmd(nc, in_maps, list(range(N_CORES)), trace=trace)
    if trace:
        kernel.last_exec_time_ns = res.exec_time_ns
        kernel.last_results = res
    # per-core out: [128, T*K*4*C] -> [T, K, BP, C]
    parts = []
    for c in range(N_CORES):
        arr = res.results[c]["out"].reshape(128, T, K, 4, C)
        parts.append(arr.transpose(1, 2, 3, 0, 4).reshape(T, K, BP, C))
    full = np.concatenate(parts, axis=2)  # [T, K, B, C]
    b_out = np.asarray(inputs["b_out"], np.float32)
    return np.ascontiguousarray(full + b_out, dtype=np.float32)


# revision 53
# speedup vs baseline: 1.7953x; 1.0186x over previous
"""Trainium2 Bass kernel for nn_Decoder (mapping MLP + hard-LSTM scan + out proj).

Self-contained: takes FULL inputs (as produced by setup_inputs), shards batch
across 8 NeuronCores, runs a Bass/Tile kernel via run_bass_kernel_spmd, and
gathers the full [T, K, B, C] output.

Layout per core (B' = B/8 = 512 batch elems):
  rows = k*B' + b  (20 "rtiles" of 512 rows each, one per k)
  h, c state: [H=128 partitions, 512 rows] bf16 tiles, one pair per rtile.

Per step & rtile (engine assignment tuned against the TRN2 cost model):
  PE   : 4 W_ih-[x;1] preloads + 4 W_hh matmuls fill two PSUM bank-pairs
         [i|g] and [f|o]; the out-projection runs TRANSPOSED (h 128-batch
         chunk as stationary, W_outT as moving) so it costs ~2 columns
         instead of 512, accumulating [128 batch, 2] slots in a PSUM bank.
  ACT  : one merged relu evac of [f|o] -> a_fo bf16; PSUM out-bank evac
         every 64 rtile-steps.
  DVE  : OP_T3 t = hs(i)*ht(g) straight from both PSUM banks;
         OP_UC u = min(a_f,1)*c; OP_H h = min(a_o,1)*ht(c) (lagged 1 rtile
         so the GPSIMD add can complete without stalling the DVE stream).
  POOL : c = u + t (native tensor add on the otherwise idle GPSIMD engine).
"""
import os
import sys

sys.path.insert(0, "/opt/trn_rl_repo")

import numpy as np
import ml_dtypes
from contextlib import ExitStack

import concourse.bass as bass
import concourse.tile as tile
from concourse import mybir, bacc
import concourse.dve_ops as _dve_ops_mod
from concourse.dve_ops import (
    DveOp, OPS, CUSTOM_DVE_SPECS, _CUSTOM_DVE_ROW_BASE, get_dve_sub_opcode,
    _COMPILE_CACHE,
)
from concourse.dve_spec import (
    Spec, Src0, Src1, C0, C1, C2, Zero, One, maxx, minn, relu, lower, _has_src1,
)
from concourse.dve_uop import (
    DveOpSpec, UopConfig, UopDpConfig, InpSel, OutSel, OutPath, AluInp,
    AluOp as UAluOp, DelayInp, Trigger,
)
from concourse.bass_utils import run_bass_kernel_spmd

FP32 = mybir.dt.float32
BF16 = mybir.dt.bfloat16
AF = mybir.ActivationFunctionType

# Full-problem config (hardcoded; the harness always calls with these shapes).
T_FULL, K_FULL, B_FULL, C_DIM, H_DIM, MH_DIM, N_CORES = 20, 20, 4096, 2, 128, 64, 8


# ---------------------------------------------------------------- custom ops
def _register_op(name, spec):
    for op in OPS:
        if op.name == name:
            return op
    shas = {}
    for ver in ("v3", "v4"):
        tmp = DveOpSpec(name=name, opcode=0, uops=lower(spec, ver=ver),
                        rd1_en=_has_src1(spec))
        shas[ver] = tmp.sha(ver)
    op = DveOp(name, spec, subdim=False, uops_sha=shas)
    OPS.append(op)
    CUSTOM_DVE_SPECS[name] = spec
    _dve_ops_mod._SUB_OPCODE_FOR_NAME[name] = _CUSTOM_DVE_ROW_BASE + len(OPS) - 1
    assert _dve_ops_mod._SUB_OPCODE_FOR_NAME[name] < 0x20
    return op


def _uop_h_2x():
    """2X_1PORT program for ANT_LSTM_H: elements lo/hi of min(Src0,1) *
    max(min(Src1,1), C2). Element 0 in blocks 0-3 (result rides delay lane 0
    to the output), element 1 in blocks 4-7 (result = block 7 ALU out).

    Input lanes (lane k feeds block 0's delay chain k-1):
      1:SRC_0->d0  2:ONE->d1  3:SRC_1->d2  4:CONST_2->d3
      5:SRC_0_HI->d4  6:SRC_1_HI->d5
    """
    u = UopConfig()
    for lane, src in ((1, InpSel.SRC_0), (2, InpSel.ONE_F32),
                      (3, InpSel.SRC_1), (4, InpSel.CONST_2),
                      (5, InpSel.SRC_0_HI), (6, InpSel.SRC_1_HI)):
        u.enable_input(src, lane)
    u.require_inp0 = 1
    u.require_inp1 = 1
    u.trigger = (Trigger.SRC_TENSOR_DONE, Trigger.NONE, Trigger.NONE)
    dp = u.datapath_config
    dp[0].enable_alu(UAluOp.MIN, AluInp.PREV_DELAY_2, AluInp.PREV_DELAY_1)
    dp[0].pass_through_delay(0, 1, 3, 4, 5)
    dp[1].enable_alu(UAluOp.MAX, AluInp.PREV_ALU_OUT, AluInp.PREV_DELAY_3)
    dp[1].pass_through_delay(0, 1, 3, 4, 5)
    dp[2].enable_alu(UAluOp.MIN, AluInp.PREV_DELAY_0, AluInp.PREV_DELAY_1)
    dp[2].enable_delay_from_src(DelayInp.PREV_ALU_OUT, 0)   # capture ht_lo
    dp[2].pass_through_delay(1, 3, 4, 5)
    dp[3].enable_alu(UAluOp.MULTIPLY, AluInp.PREV_ALU_OUT, AluInp.PREV_DELAY_0)
    dp[3].pass_through_delay(1, 3, 4, 5)
    dp[4].enable_alu(UAluOp.MIN, AluInp.PREV_DELAY_5, AluInp.PREV_DELAY_1)
    dp[4].enable_delay_from_src(DelayInp.PREV_ALU_OUT, 0)   # capture r_lo
    dp[4].pass_through_delay(1, 3, 4)
    dp[5].enable_alu(UAluOp.MAX, AluInp.PREV_ALU_OUT, AluInp.PREV_DELAY_3)
    dp[5].pass_through_delay(0, 1, 4)
    dp[6].enable_alu(UAluOp.MIN, AluInp.PREV_DELAY_4, AluInp.PREV_DELAY_1)
    dp[6].enable_delay_from_src(DelayInp.PREV_ALU_OUT, 2)   # capture ht_hi
    dp[6].pass_through_delay(0)
    dp[7].enable_alu(UAluOp.MULTIPLY, AluInp.PREV_ALU_OUT, AluInp.PREV_DELAY_2)
    dp[7].pass_through_delay(0)
    u.enable_output(OutSel.DELAY_0, OutPath.WR0_LO)
    u.enable_output(OutSel.ALU_OUT, OutPath.WR0_HI)
    return [u]


def _uop_uc_2x():
    """2X_1PORT program for ANT_LSTM_UC: lo/hi of min(Src0,1) * Src1.
    Lanes: 1:SRC_0->d0 2:ONE->d1 3:SRC_1->d2 4:SRC_0_HI->d3 5:SRC_1_HI->d4."""
    u = UopConfig()
    for lane, src in ((1, InpSel.SRC_0), (2, InpSel.ONE_F32),
                      (3, InpSel.SRC_1), (4, InpSel.SRC_0_HI),
                      (5, InpSel.SRC_1_HI)):
        u.enable_input(src, lane)
    u.require_inp0 = 1
    u.require_inp1 = 1
    u.trigger = (Trigger.SRC_TENSOR_DONE, Trigger.NONE, Trigger.NONE)
    dp = u.datapath_config
    dp[0].enable_alu(UAluOp.MIN, AluInp.PREV_DELAY_0, AluInp.PREV_DELAY_1)
    dp[0].pass_through_delay(1, 2, 3, 4)
    dp[1].enable_alu(UAluOp.MULTIPLY, AluInp.PREV_ALU_OUT, AluInp.PREV_DELAY_2)
    dp[1].pass_through_delay(1, 3, 4)
    dp[2].enable_alu(UAluOp.MIN, AluInp.PREV_DELAY_3, AluInp.PREV_DELAY_1)
    dp[2].enable_delay_from_src(DelayInp.PREV_ALU_OUT, 0)   # capture r_lo
    dp[2].pass_through_delay(4)
    dp[3].enable_alu(UAluOp.MULTIPLY, AluInp.PREV_ALU_OUT, AluInp.PREV_DELAY_4)
    dp[3].pass_through_delay(0)
    for b in range(4, 8):
        dp[b].pass_through_alu()
        dp[b].pass_through_delay(0)
    u.enable_output(OutSel.DELAY_0, OutPath.WR0_LO)
    u.enable_output(OutSel.ALU_OUT, OutPath.WR0_HI)
    return [u]


def _enable_2x(op, uops_2x):
    """Pre-populate the DveOp compile cache with a spec carrying a 2X_1PORT
    uop program so dve_table_for_ops writes the mode entries. Instructions
    opt in per call site via ins.perf_max = 1 (the engine still falls back
    to 1x at runtime when operands are not packed 2-byte)."""
    for ver in ("v3", "v4"):
        spec = DveOpSpec(
            name=op.name,
            opcode=get_dve_sub_opcode(op.name),
            uops=lower(op.spec, ver=ver),
            rd1_en=_has_src1(op.spec),
            uops_2x=uops_2x,
            perf_max=1,
        )
        _COMPILE_CACHE[(op.name, ver)] = spec


def _hs(x, s0, s1):
    return np.minimum(np.maximum(x * s0 + s1, 0.0), 1.0)


def _ht(x, lo):
    return np.maximum(np.minimum(x, 1.0), lo)


# u = min(a_f, 1) * c ; a_f = relu(f') from ACT
OP_UC = _register_op(
    "ANT_LSTM_UC",
    Spec(body=minn(Src0, One) * Src1,
         reference=lambda in0, in1, s0, s1, imm2: np.minimum(in0, 1.0) * in1),
)
# u = hs(f') * c with f' read straight from PSUM (relu+min in-op)
OP_UF = _register_op(
    "ANT_LSTM_UF",
    Spec(body=minn(relu(Src0), One) * Src1,
         reference=lambda in0, in1, s0, s1, imm2:
             np.minimum(np.maximum(in0, 0.0), 1.0) * in1),
)
# h = min(a_o,1)*clip(c, -1, 1); a_o = relu(o') from ACT; imm2=-1.
# Also used for t = min(a_i,1)*ht(g) with in1 = the raw g-gate PSUM bank
# (DVE may read at most one non-scalar operand from PSUM).
OP_H = _register_op(
    "ANT_LSTM_H",
    Spec(body=minn(Src0, One) * maxx(minn(Src1, One), C2),
         reference=lambda in0, in1, s0, s1, imm2:
             np.minimum(in0, 1.0) * np.maximum(np.minimum(in1, 1.0), imm2)),
)
# leaky_relu(psum + bm1) = max(y, 0.01*y), y = Src0 + C1; imm2 = slope
OP_LRELU = _register_op(
    "ANT_LRELU",
    Spec(body=maxx(Src0 + C1, (Src0 + C1) * C2),
         reference=lambda in0, in1, s0, s1, imm2:
             np.maximum(in0 + s1, (in0 + s1) * imm2)),
)

_enable_2x(OP_H, _uop_h_2x())
_enable_2x(OP_UC, _uop_uc_2x())


# ---------------------------------------------------------------- bass build
def build_nc(T, K, BP, use_pack=False, repeat=1, tmp_bufs=6,
             c_add_eng="vector", out_lag=2, h_lag=2, evac_mode="fo_merged",
             pair_mode="ig_fo", out_evac_eng="vector", out_pos="batch4",
             **_unused):
    """Build the per-core Bass program. BP = per-core batch (must be 512).

    PSUM banks (8 x 512 fp32): pa = [i | g] x2 bufs (DVE-read), pb = [f | o]
    x1 buf (ACT-evac'd early), po = out-projection slot banks x2 bufs.
    Biases + hardsigmoid pre-scale folded into the augmented W_ih-[x;1]
    preload on the host: i/f/o psum arrive as (raw+b)/6+0.5, g as raw+b.
    """
    H, MH, C = H_DIM, MH_DIM, C_DIM
    CA = C + 1        # augmented x rows (x0, x1, 1)
    FD = BP           # free dim of every tile
    PSB = 512         # PSUM bank stride in fp32 elems
    RT = K            # rtiles per step
    assert FD == PSB, "layout assumes BP == 512"
    nc = bacc.Bacc("TRN2", target_bir_lowering=False, debug=False)

    phT_e = nc.declare_dram_parameter("phT", [H, K * BP], BF16, isOutput=False)
    xh_e = nc.declare_dram_parameter("xh", [CA, T * BP], BF16, isOutput=False)
    whhT_e = nc.declare_dram_parameter("whhT", [H, 4 * H], BF16, isOutput=False)
    wihT_e = nc.declare_dram_parameter("wihT", [CA, 4 * H], BF16, isOutput=False)
    woutT_e = nc.declare_dram_parameter("woutT", [H, C], BF16, isOutput=False)
    wm1T_e = nc.declare_dram_parameter("wm1T", [H, MH], BF16, isOutput=False)
    wm2T_e = nc.declare_dram_parameter("wm2T", [MH, H], BF16, isOutput=False)
    biasp_e = nc.declare_dram_parameter("biasp", [H, 2], FP32, isOutput=False)
    # out: [128 batch-chunk partitions, T*K*4chunks*2coords] fp32
    NOUT = T * K * 4 * C
    out_e = nc.declare_dram_parameter("out", [128, NOUT], FP32, isOutput=True)

    with tile.TileContext(nc) as tc:
        with ExitStack() as ctx:
            wts = ctx.enter_context(tc.tile_pool(name="wts", bufs=1))
            big = ctx.enter_context(tc.tile_pool(name="big", bufs=1))
            st = ctx.enter_context(tc.tile_pool(name="st", bufs=1))
            tmp = ctx.enter_context(tc.tile_pool(name="tmp", bufs=tmp_bufs))
            psA = ctx.enter_context(tc.tile_pool(name="psA", bufs=2, space="PSUM"))
            psB = ctx.enter_context(tc.tile_pool(name="psB", bufs=2, space="PSUM"))

            # ---- weights / constants into SBUF
            whhT = wts.tile([H, 4 * H], BF16, tag="whhT")
            nc.sync.dma_start(whhT[:], whhT_e[:])
            woutT = wts.tile([H, C], BF16, tag="woutT")
            nc.sync.dma_start(woutT[:], woutT_e[:])
            wm1T = wts.tile([H, MH], BF16, tag="wm1T")
            nc.sync.dma_start(wm1T[:], wm1T_e[:])
            wm2T = wts.tile([MH, H], BF16, tag="wm2T")
            nc.sync.dma_start(wm2T[:], wm2T_e[:])
            biasp = wts.tile([H, 2], FP32, tag="biasp")
            nc.sync.dma_start(biasp[:], biasp_e[:])

            npack = 4 if use_pack else 1
            nprows = 32 * (npack - 1) + CA
            wih = wts.tile([nprows, 4 * H], BF16, tag="wih")
            xrep = wts.tile([nprows, T * BP], BF16, tag="xrep")
            phT = big.tile([H, K * BP], BF16, tag="phT")
            HK = K * BP // 2
            # first halves first: MLP(0) / step-0 gates wait ~half the DMA
            nc.sync.dma_start(phT[:, 0:HK], phT_e[:, 0:HK])
            for r in range(npack):
                nc.sync.dma_start(wih[32 * r:32 * r + CA, :], wihT_e[:])
                nc.sync.dma_start(xrep[32 * r:32 * r + CA, 0:BP],
                                  xh_e[:, 0:BP])
                nc.sync.dma_start(xrep[32 * r:32 * r + CA, BP:T * BP],
                                  xh_e[:, BP:T * BP])
            nc.sync.dma_start(phT[:, HK:], phT_e[:, HK:])

            # SBUF accumulation buffer for the final output
            out_sb = big.tile([128, NOUT], FP32, tag="out_sb")

            # ---- persistent state tiles
            h_t = [st.tile([H, FD], BF16, name=f"h{j}", tag=f"h{j}")
                   for j in range(RT)]
            c_t = [st.tile([H, FD], BF16, name=f"c{j}", tag=f"c{j}")
                   for j in range(RT)]

            # ---- mapping MLP -> h0 (interleaved into the first step's
            # pipeline: MLP(j) shares the psA pool rotation and is emitted
            # two rtiles ahead of the gate matmuls that consume h_t[j])
            def emit_mlp(j):
                pm = psA.tile([H, 2 * PSB], FP32, tag="pa", name="pm")
                nc.tensor.matmul(pm[0:MH, 0:FD], wm1T[:, 0:MH],
                                 phT[:, j * FD:(j + 1) * FD],
                                 start=True, stop=True)
                a1 = tmp.tile([MH, FD], BF16, tag="a1")
                nc.vector._custom_dve(OP_LRELU, out=a1[:], in0=pm[0:MH, 0:FD],
                                      s1=biasp[0:MH, 0:1], imm2=0.01)
                nc.tensor.matmul(pm[0:H, PSB:PSB + FD], wm2T[:, 0:H], a1[:],
                                 start=True, stop=True)
                nc.scalar.activation(h_t[j][:], pm[0:H, PSB:PSB + FD],
                                     AF.Identity, bias=biasp[:, 1:2], scale=1.0)

            for j in range(RT):
                emit_mlp(j)

            # gate chunk offsets in whhT / wih cols: i=0, f=1, g=2, o=3
            CH = {"i": 0, "f": 1, "g": 2, "o": 3}

            def gcol(name):
                m = CH[name]
                return slice(m * H, (m + 1) * H)

            def gates_mm(ps, xcols, j, names, slices=None):
                # W_ih-[x;1] preloads (contract=3, row-packed), then W_hh accum
                if slices is None:
                    slices = [slice(0, FD) if r == 0 else slice(PSB, PSB + FD)
                              for r in range(len(names))]
                for gname, sl in zip(names, slices):
                    rr = (CH[gname] if use_pack else 0)
                    nc.tensor.matmul(
                        ps[:, sl],
                        wih[32 * rr:32 * rr + CA, gcol(gname)],
                        xrep[32 * rr:32 * rr + CA, xcols],
                        start=True, stop=False,
                        tile_position=(32 * rr, 0) if use_pack else None,
                    )
                for gname, sl in zip(names, slices):
                    nc.tensor.matmul(ps[:, sl], whhT[:, gcol(gname)],
                                     h_t[j][:], start=False, stop=True)

            # ---- flat software-pipelined main loop over s = tl*RT + j.
            # The [i|g] pa-group runs ONE rtile ahead of the [f|o] pb-group so
            # the PE has ~1.7us of queued work inside the a_fo ACT-evac window
            # (pb is single-buffered; its bank is reusable only after a_fo).
            S = T * repeat * RT
            pa_tiles = {}   # s -> (pa_tile, a_i_tile)
            afo_tiles = {}  # s -> a_fo_tile
            pend_h = []     # s queued for OP_H (lagged h_lag rtiles)
            pend_out = []   # s queued for out-proj (lagged out_lag rtiles)

            def s_tj(s):
                return (s // RT) % T, s % RT

            def emit_pa(s):
                t, j = s_tj(s)
                if pair_mode == "ig_fo":
                    pa = psA.tile([H, 2 * PSB], FP32, tag="pa")  # [i | g]
                    gates_mm(pa, slice(t * BP, (t + 1) * BP), j, "ig")
                    a_i = tmp.tile([H, FD], BF16, tag="ai")
                    nc.scalar.activation(a_i[:], pa[:, 0:FD], AF.Relu,
                                         bias=0.0, scale=1.0)
                    pa_tiles[s] = (pa, a_i[:])
                else:  # gf_io: both halves consumed by DVE straight from psum
                    pa = psA.tile([H, 2 * PSB], FP32, tag="pa")  # [g | f]
                    gates_mm(pa, slice(t * BP, (t + 1) * BP), j, "gf")
                    pa_tiles[s] = (pa, None)

            def emit_h(s):
                t, j = s_tj(s)
                a_o_ap = afo_tiles.pop(s)
                r = nc.vector._custom_dve(OP_H, out=h_t[j][:], in0=a_o_ap,
                                          in1=c_t[j][:], imm2=-1.0)
                r.ins.perf_max = 1   # all-bf16 SBUF -> 2X_1PORT eligible

            def emit_out(s, host_pa, col_off=0, evac=True, evac_cols=8):
                # Transposed out-projection: h 128-batch chunk is stationary,
                # woutT moving -> [128 batch, 2] psum slots. The slots land in
                # the DEAD tail of `host_pa`'s g-bank (already consumed by
                # OP_H-as-t), so no dedicated psum bank is needed and psB can
                # double-buffer. Evac'd to out_sb (8 fp32 per rtile-step).
                t, j = s_tj(s)
                sm = s % (T * RT)
                gend = 2 * PSB if pair_mode == "ig_fo" else PSB
                base = gend - 8 - col_off
                for q in range(4):
                    nc.tensor.matmul(host_pa[:, base + C * q:base + C * (q + 1)],
                                     h_t[j][:, 128 * q:128 * (q + 1)],
                                     woutT[:, 0:C], start=True, stop=True)
                if not evac:
                    return
                elo = base + 8 - evac_cols
                start = (sm + 1) * 8 - evac_cols
                eng = out_evac_eng
                if eng == "alt":
                    eng = "scalar" if (sm // 2) % 2 else "vector"
                if eng == "scalar":
                    nc.scalar.activation(out_sb[:, start:start + evac_cols],
                                         host_pa[:, elo:elo + evac_cols],
                                         AF.Copy, bias=0.0, scale=1.0)
                else:
                    nc.vector.tensor_scalar_add(
                        out_sb[:, start:start + evac_cols],
                        host_pa[:, elo:elo + evac_cols], 0.0)
                # stream completed 512-col out_sb regions to DRAM during the
                # loop so no big DMA sits in the drain tail (last repeat only)
                if s >= S - T * RT:
                    end = start + evac_cols
                    if end % 512 == 0 or end == NOUT:
                        lo = (end - 1) // 512 * 512
                        nc.sync.dma_start(out_e[:, lo:end], out_sb[:, lo:end])

            emit_pa(0)
            prev_pa = None
            for s in range(S):
                t, j = s_tj(s)
                if out_pos == "early" and prev_pa is not None \
                        and len(pend_out) > out_lag:
                    emit_out(pend_out.pop(0), prev_pa)
                pb = psB.tile([H, 2 * PSB], FP32, tag="pb")
                if pair_mode == "ig_fo":
                    xc = slice(t * BP, (t + 1) * BP)
                    if t == 0:   # f gate (and its 2 matmuls) dead when c==0
                        gates_mm(pb, xc, j, "o",
                                 slices=[slice(PSB, PSB + FD)])
                    else:
                        gates_mm(pb, xc, j, "fo")
                    branch = (("o_only" if s % 2 else "fo_merged")
                              if evac_mode == "alt" else evac_mode)
                    if t == 0:
                        branch = "o_only"   # f gate unused when c==0
                    if branch == "o_only":
                        a_o = tmp.tile([H, FD], BF16, tag="ao")
                        nc.scalar.activation(a_o[:], pb[:, PSB:PSB + FD],
                                             AF.Relu, bias=0.0, scale=1.0)
                        afo_tiles[s] = a_o[:]
                        if t > 0:
                            u_d = tmp.tile([H, FD], BF16, tag="uu")
                            nc.vector._custom_dve(OP_UF, out=u_d[:],
                                                  in0=pb[:, 0:FD],
                                                  in1=c_t[j][:])
                    else:
                        a_fo = tmp.tile([H, 2 * PSB], BF16, tag="afo")
                        nc.scalar.activation(a_fo[:], pb[:, 0:2 * PSB],
                                             AF.Relu, bias=0.0, scale=1.0)
                        afo_tiles[s] = a_fo[:, PSB:PSB + FD]
                        if t > 0:
                            u_d = tmp.tile([H, FD], BF16, tag="uu")
                            r = nc.vector._custom_dve(OP_UC, out=u_d[:],
                                                      in0=a_fo[:, 0:FD],
                                                      in1=c_t[j][:])
                            r.ins.perf_max = 1
                else:  # gf_io: pb = [i | o], one merged ACT evac covers both
                    gates_mm(pb, slice(t * BP, (t + 1) * BP), j, "io")
                    a_io = tmp.tile([H, 2 * PSB], BF16, tag="afo")
                    nc.scalar.activation(a_io[:], pb[:, 0:2 * PSB], AF.Relu,
                                         bias=0.0, scale=1.0)
                    afo_tiles[s] = a_io[:, PSB:PSB + FD]   # a_o
                if s + 1 < S:
                    emit_pa(s + 1)   # prefetched pa pair for the next rtile

                # DVE: t = min(a_i,1)*ht(g); g raw+b read from psum.
                # The g-bank tail (last 8 cols) doubles as the out-proj slot
                # region after this read (WAR-sem'd by the tile pool).
                pa, a_i = pa_tiles.pop(s)
                if pair_mode == "gf_io":
                    a_i = a_io[:, 0:FD]
                    g_ap = pa[:, 0:FD]
                    if t > 0:
                        u_d = tmp.tile([H, FD], BF16, tag="uu")
                        nc.vector._custom_dve(OP_UF, out=u_d[:],
                                              in0=pa[:, PSB:PSB + FD],
                                              in1=c_t[j][:])
                else:
                    g_ap = pa[:, PSB:PSB + FD]
                if t == 0:
                    t_dst = c_t[j]      # c0 = 0 -> c1 = t
                else:
                    t_dst = tmp.tile([H, FD], BF16, tag="tt")
                nc.vector._custom_dve(OP_H, out=t_dst[:], in0=a_i,
                                      in1=g_ap, imm2=-1.0)
                # lagged OP_H (waits out the GPSIMD add latency off-stream)
                if len(pend_h) >= h_lag:
                    emit_h(pend_h.pop(0))
                if t > 0:
                    add_e = getattr(nc, c_add_eng)
                    add_e.tensor_add(c_t[j][:], u_d[:], t_dst[:])
                pend_h.append(s)
                if j == RT - 1:      # step boundary: drain so h is final
                    while pend_h:
                        emit_h(pend_h.pop(0))

                pend_out.append(s)
                if out_pos == "late" and len(pend_out) > out_lag:
                    emit_out(pend_out.pop(0), pa)
                elif out_pos == "batch4" and len(pend_out) > out_lag + 3 \
                        and s % 4 == 3:
                    sms = [p % (T * RT) for p in pend_out[:4]]
                    if sms[0] + 3 == sms[3]:
                        for k in range(3):
                            emit_out(pend_out.pop(0), pa, col_off=8 * (3 - k),
                                     evac=False)
                        emit_out(pend_out.pop(0), pa, col_off=0, evac=True,
                                 evac_cols=32)
                    else:
                        for k in range(4):
                            emit_out(pend_out.pop(0), pa, col_off=8 * (3 - k))
                elif out_pos == "batch2" and len(pend_out) > out_lag + 1 \
                        and s % 2 == 1:
                    # two lagged out-projs share one host tile + one evac
                    s0, s1 = pend_out[0], pend_out[1]
                    if s0 % (T * RT) + 1 == s1 % (T * RT):
                        emit_out(pend_out.pop(0), pa, col_off=8, evac=False)
                        emit_out(pend_out.pop(0), pa, col_off=0, evac=True,
                                 evac_cols=16)
                    else:   # repeat-boundary wrap: evac singly
                        emit_out(pend_out.pop(0), pa, col_off=8)
                        emit_out(pend_out.pop(0), pa, col_off=0)
                prev_pa = pa
            last_pa = pa
            for k in range(len(pend_out)):
                emit_out(pend_out[k], last_pa, col_off=8 * (k + 1))
            pend_out.clear()

    nc.finalize()
    return nc


# ---------------------------------------------------------------- host side
def _bf16(x):
    return np.ascontiguousarray(x, dtype=np.float32).astype(ml_dtypes.bfloat16)


def prep_core_inputs(inputs, core, T, K, BP):
    H, MH, C = H_DIM, MH_DIM, C_DIM
    b0 = core * BP
    ph = np.asarray(inputs["pred_lstm_hidden"], np.float32)[:, b0:b0 + BP, :]
    phT = ph.transpose(2, 0, 1).reshape(H, K * BP)
    idx = np.concatenate([[0], np.arange(T - 1)])
    obs = np.asarray(inputs["obs_traj_rel"], np.float32)
    xs = obs[idx][:, b0:b0 + BP, :C]
    xh = xs.transpose(2, 0, 1).reshape(C, T * BP)
    xh = np.concatenate([xh, np.ones((1, T * BP), np.float32)], axis=0)
    bsum = (np.asarray(inputs["b_ih"], np.float32)
            + np.asarray(inputs["b_hh"], np.float32))
    # per-gate-chunk scale and bias folded into W_hh / W_ih / the x=1 row:
    #   i, f, o chunks: psum = (raw + b)/6 + 0.5 ; g chunk: psum = raw + b
    scale = np.ones(4 * H, np.float32) / 6.0
    scale[2 * H:3 * H] = 1.0
    bias_row = bsum * scale
    bias_row[0:2 * H] += 0.5
    bias_row[3 * H:4 * H] += 0.5
    whh_s = np.asarray(inputs["W_hh"], np.float32) * scale[:, None]
    wih_s = np.asarray(inputs["W_ih"], np.float32) * scale[:, None]
    wih_aug = np.concatenate([wih_s.T, bias_row[None, :]], axis=0)  # [3, 4H]
    biasp = np.zeros((H, 2), np.float32)
    biasp[0:MH, 0] = np.asarray(inputs["bm1"], np.float32)
    biasp[:, 1] = np.asarray(inputs["bm2"], np.float32)
    return {
        "phT": _bf16(phT),
        "xh": _bf16(xh),
        "whhT": _bf16(whh_s.T),
        "wihT": _bf16(wih_aug),
        "woutT": _bf16(np.asarray(inputs["W_out"], np.float32).T),
        "wm1T": _bf16(np.asarray(inputs["Wm1"], np.float32).T),
        "wm2T": _bf16(np.asarray(inputs["Wm2"], np.float32).T),
        "biasp": biasp,
    }


_NC_CACHE = {}


def _get_nc(T, K, BP):
    key = (T, K, BP)
    if key not in _NC_CACHE:
        _NC_CACHE[key] = build_nc(T, K, BP)
    return _NC_CACHE[key]


def kernel(**inputs) -> np.ndarray:
    T, K, B, C = T_FULL, K_FULL, B_FULL, C_DIM
    BP = B // N_CORES
    nc = _get_nc(T, K, BP)
    in_maps = [prep_core_inputs(inputs, c, T, K, BP) for c in range(N_CORES)]
    trace = bool(int(os.environ.get("KERNEL_TRACE", "0")))
    res = run_bass_kernel_spmd(nc, in_maps, list(range(N_CORES)), trace=trace)
    if trace:
        kernel.last_exec_time_ns = res.exec_time_ns
        kernel.last_results = res
    # per-core out: [128, T*K*4*C] -> [T, K, BP, C]
    parts = []
    for c in range(N_CORES):
        arr = res.results[c]["out"].reshape(128, T, K, 4, C)
        parts.append(arr.transpose(1, 2, 3, 0, 4).reshape(T, K, BP, C))
    full = np.concatenate(parts, axis=2)  # [T, K, B, C]
    b_out = np.asarray(inputs["b_out"], np.float32)
    return np.ascontiguousarray(full + b_out, dtype=np.float32)


# revision 59
# speedup vs baseline: 1.8183x; 1.0128x over previous
"""Trainium2 Bass kernel for nn_Decoder (mapping MLP + hard-LSTM scan + out proj).

Self-contained: takes FULL inputs (as produced by setup_inputs), shards batch
across 8 NeuronCores, runs a Bass/Tile kernel via run_bass_kernel_spmd, and
gathers the full [T, K, B, C] output.

Layout per core (B' = B/8 = 512 batch elems):
  rows = k*B' + b  (20 "rtiles" of 512 rows each, one per k)
  h, c state: [H=128 partitions, 512 rows] bf16 tiles, one pair per rtile.

Per step & rtile (engine assignment tuned against the TRN2 cost model):
  PE   : 4 W_ih-[x;1] preloads + 4 W_hh matmuls fill two PSUM bank-pairs
         [i|g] and [f|o]; the out-projection runs TRANSPOSED (h 128-batch
         chunk as stationary, W_outT as moving) so it costs ~2 columns
         instead of 512, accumulating [128 batch, 2] slots in a PSUM bank.
  ACT  : one merged relu evac of [f|o] -> a_fo bf16; PSUM out-bank evac
         every 64 rtile-steps.
  DVE  : OP_T3 t = hs(i)*ht(g) straight from both PSUM banks;
         OP_UC u = min(a_f,1)*c; OP_H h = min(a_o,1)*ht(c) (lagged 1 rtile
         so the GPSIMD add can complete without stalling the DVE stream).
  POOL : c = u + t (native tensor add on the otherwise idle GPSIMD engine).
"""
import os
import sys

sys.path.insert(0, "/opt/trn_rl_repo")

import numpy as np
import ml_dtypes
from contextlib import ExitStack

import concourse.bass as bass
import concourse.tile as tile
from concourse import mybir, bacc
import concourse.dve_ops as _dve_ops_mod
from concourse.dve_ops import (
    DveOp, OPS, CUSTOM_DVE_SPECS, _CUSTOM_DVE_ROW_BASE, get_dve_sub_opcode,
    _COMPILE_CACHE,
)
from concourse.dve_spec import (
    Spec, Src0, Src1, C0, C1, C2, Zero, One, maxx, minn, relu, lower, _has_src1,
)
from concourse.dve_uop import (
    DveOpSpec, UopConfig, UopDpConfig, InpSel, OutSel, OutPath, AluInp,
    AluOp as UAluOp, DelayInp, Trigger,
)
from concourse.bass_utils import run_bass_kernel_spmd

FP32 = mybir.dt.float32
BF16 = mybir.dt.bfloat16
AF = mybir.ActivationFunctionType

# Full-problem config (hardcoded; the harness always calls with these shapes).
T_FULL, K_FULL, B_FULL, C_DIM, H_DIM, MH_DIM, N_CORES = 20, 20, 4096, 2, 128, 64, 8


# ---------------------------------------------------------------- custom ops
def _register_op(name, spec):
    for op in OPS:
        if op.name == name:
            return op
    shas = {}
    for ver in ("v3", "v4"):
        tmp = DveOpSpec(name=name, opcode=0, uops=lower(spec, ver=ver),
                        rd1_en=_has_src1(spec))
        shas[ver] = tmp.sha(ver)
    op = DveOp(name, spec, subdim=False, uops_sha=shas)
    OPS.append(op)
    CUSTOM_DVE_SPECS[name] = spec
    _dve_ops_mod._SUB_OPCODE_FOR_NAME[name] = _CUSTOM_DVE_ROW_BASE + len(OPS) - 1
    assert _dve_ops_mod._SUB_OPCODE_FOR_NAME[name] < 0x20
    return op


def _uop_h_2x():
    """2X_1PORT program for ANT_LSTM_H: elements lo/hi of min(Src0,1) *
    max(min(Src1,1), C2). Element 0 in blocks 0-3 (result rides delay lane 0
    to the output), element 1 in blocks 4-7 (result = block 7 ALU out).

    Input lanes (lane k feeds block 0's delay chain k-1):
      1:SRC_0->d0  2:ONE->d1  3:SRC_1->d2  4:CONST_2->d3
      5:SRC_0_HI->d4  6:SRC_1_HI->d5
    """
    u = UopConfig()
    for lane, src in ((1, InpSel.SRC_0), (2, InpSel.ONE_F32),
                      (3, InpSel.SRC_1), (4, InpSel.CONST_2),
                      (5, InpSel.SRC_0_HI), (6, InpSel.SRC_1_HI)):
        u.enable_input(src, lane)
    u.require_inp0 = 1
    u.require_inp1 = 1
    u.trigger = (Trigger.SRC_TENSOR_DONE, Trigger.NONE, Trigger.NONE)
    dp = u.datapath_config
    dp[0].enable_alu(UAluOp.MIN, AluInp.PREV_DELAY_2, AluInp.PREV_DELAY_1)
    dp[0].pass_through_delay(0, 1, 3, 4, 5)
    dp[1].enable_alu(UAluOp.MAX, AluInp.PREV_ALU_OUT, AluInp.PREV_DELAY_3)
    dp[1].pass_through_delay(0, 1, 3, 4, 5)
    dp[2].enable_alu(UAluOp.MIN, AluInp.PREV_DELAY_0, AluInp.PREV_DELAY_1)
    dp[2].enable_delay_from_src(DelayInp.PREV_ALU_OUT, 0)   # capture ht_lo
    dp[2].pass_through_delay(1, 3, 4, 5)
    dp[3].enable_alu(UAluOp.MULTIPLY, AluInp.PREV_ALU_OUT, AluInp.PREV_DELAY_0)
    dp[3].pass_through_delay(1, 3, 4, 5)
    dp[4].enable_alu(UAluOp.MIN, AluInp.PREV_DELAY_5, AluInp.PREV_DELAY_1)
    dp[4].enable_delay_from_src(DelayInp.PREV_ALU_OUT, 0)   # capture r_lo
    dp[4].pass_through_delay(1, 3, 4)
    dp[5].enable_alu(UAluOp.MAX, AluInp.PREV_ALU_OUT, AluInp.PREV_DELAY_3)
    dp[5].pass_through_delay(0, 1, 4)
    dp[6].enable_alu(UAluOp.MIN, AluInp.PREV_DELAY_4, AluInp.PREV_DELAY_1)
    dp[6].enable_delay_from_src(DelayInp.PREV_ALU_OUT, 2)   # capture ht_hi
    dp[6].pass_through_delay(0)
    dp[7].enable_alu(UAluOp.MULTIPLY, AluInp.PREV_ALU_OUT, AluInp.PREV_DELAY_2)
    dp[7].pass_through_delay(0)
    u.enable_output(OutSel.DELAY_0, OutPath.WR0_LO)
    u.enable_output(OutSel.ALU_OUT, OutPath.WR0_HI)
    return [u]


def _uop_uc_2x():
    """2X_1PORT program for ANT_LSTM_UC: lo/hi of min(Src0,1) * Src1.
    Lanes: 1:SRC_0->d0 2:ONE->d1 3:SRC_1->d2 4:SRC_0_HI->d3 5:SRC_1_HI->d4."""
    u = UopConfig()
    for lane, src in ((1, InpSel.SRC_0), (2, InpSel.ONE_F32),
                      (3, InpSel.SRC_1), (4, InpSel.SRC_0_HI),
                      (5, InpSel.SRC_1_HI)):
        u.enable_input(src, lane)
    u.require_inp0 = 1
    u.require_inp1 = 1
    u.trigger = (Trigger.SRC_TENSOR_DONE, Trigger.NONE, Trigger.NONE)
    dp = u.datapath_config
    dp[0].enable_alu(UAluOp.MIN, AluInp.PREV_DELAY_0, AluInp.PREV_DELAY_1)
    dp[0].pass_through_delay(1, 2, 3, 4)
    dp[1].enable_alu(UAluOp.MULTIPLY, AluInp.PREV_ALU_OUT, AluInp.PREV_DELAY_2)
    dp[1].pass_through_delay(1, 3, 4)
    dp[2].enable_alu(UAluOp.MIN, AluInp.PREV_DELAY_3, AluInp.PREV_DELAY_1)
    dp[2].enable_delay_from_src(DelayInp.PREV_ALU_OUT, 0)   # capture r_lo
    dp[2].pass_through_delay(4)
    dp[3].enable_alu(UAluOp.MULTIPLY, AluInp.PREV_ALU_OUT, AluInp.PREV_DELAY_4)
    dp[3].pass_through_delay(0)
    for b in range(4, 8):
        dp[b].pass_through_alu()
        dp[b].pass_through_delay(0)
    u.enable_output(OutSel.DELAY_0, OutPath.WR0_LO)
    u.enable_output(OutSel.ALU_OUT, OutPath.WR0_HI)
    return [u]


def _enable_2x(op, uops_2x):
    """Pre-populate the DveOp compile cache with a spec carrying a 2X_1PORT
    uop program so dve_table_for_ops writes the mode entries. Instructions
    opt in per call site via ins.perf_max = 1 (the engine still falls back
    to 1x at runtime when operands are not packed 2-byte)."""
    for ver in ("v3", "v4"):
        spec = DveOpSpec(
            name=op.name,
            opcode=get_dve_sub_opcode(op.name),
            uops=lower(op.spec, ver=ver),
            rd1_en=_has_src1(op.spec),
            uops_2x=uops_2x,
            perf_max=1,
        )
        _COMPILE_CACHE[(op.name, ver)] = spec


def _hs(x, s0, s1):
    return np.minimum(np.maximum(x * s0 + s1, 0.0), 1.0)


def _ht(x, lo):
    return np.maximum(np.minimum(x, 1.0), lo)


# u = min(a_f, 1) * c ; a_f = relu(f') from ACT
OP_UC = _register_op(
    "ANT_LSTM_UC",
    Spec(body=minn(Src0, One) * Src1,
         reference=lambda in0, in1, s0, s1, imm2: np.minimum(in0, 1.0) * in1),
)
# u = hs(f') * c with f' read straight from PSUM (relu+min in-op)
OP_UF = _register_op(
    "ANT_LSTM_UF",
    Spec(body=minn(relu(Src0), One) * Src1,
         reference=lambda in0, in1, s0, s1, imm2:
             np.minimum(np.maximum(in0, 0.0), 1.0) * in1),
)
# h = min(a_o,1)*clip(c, -1, 1); a_o = relu(o') from ACT; imm2=-1.
# Also used for t = min(a_i,1)*ht(g) with in1 = the raw g-gate PSUM bank
# (DVE may read at most one non-scalar operand from PSUM).
OP_H = _register_op(
    "ANT_LSTM_H",
    Spec(body=minn(Src0, One) * maxx(minn(Src1, One), C2),
         reference=lambda in0, in1, s0, s1, imm2:
             np.minimum(in0, 1.0) * np.maximum(np.minimum(in1, 1.0), imm2)),
)
# leaky_relu(psum + bm1) = max(y, 0.01*y), y = Src0 + C1; imm2 = slope
OP_LRELU = _register_op(
    "ANT_LRELU",
    Spec(body=maxx(Src0 + C1, (Src0 + C1) * C2),
         reference=lambda in0, in1, s0, s1, imm2:
             np.maximum(in0 + s1, (in0 + s1) * imm2)),
)

_enable_2x(OP_H, _uop_h_2x())
_enable_2x(OP_UC, _uop_uc_2x())


# ---------------------------------------------------------------- bass build
def build_nc(T, K, BP, use_pack=False, repeat=1, tmp_bufs=6,
             c_add_eng="vector", out_lag=2, h_lag=2, evac_mode="fo_merged",
             pair_mode="ig_fo", out_evac_eng="vector", out_pos="batch4",
             **_unused):
    """Build the per-core Bass program. BP = per-core batch (must be 512).

    PSUM banks (8 x 512 fp32): pa = [i | g] x2 bufs (DVE-read), pb = [f | o]
    x1 buf (ACT-evac'd early), po = out-projection slot banks x2 bufs.
    Biases + hardsigmoid pre-scale folded into the augmented W_ih-[x;1]
    preload on the host: i/f/o psum arrive as (raw+b)/6+0.5, g as raw+b.
    """
    H, MH, C = H_DIM, MH_DIM, C_DIM
    CA = C + 1        # augmented x rows (x0, x1, 1)
    FD = BP           # free dim of every tile
    PSB = 512         # PSUM bank stride in fp32 elems
    RT = K            # rtiles per step
    assert FD == PSB, "layout assumes BP == 512"
    nc = bacc.Bacc("TRN2", target_bir_lowering=False, debug=False)

    phT_e = nc.declare_dram_parameter("phT", [H, K * BP], BF16, isOutput=False)
    xh_e = nc.declare_dram_parameter("xh", [CA, T * BP], BF16, isOutput=False)
    whhT_e = nc.declare_dram_parameter("whhT", [H, 4 * H], BF16, isOutput=False)
    wihT_e = nc.declare_dram_parameter("wihT", [CA, 4 * H], BF16, isOutput=False)
    woutT_e = nc.declare_dram_parameter("woutT", [H, C], BF16, isOutput=False)
    wm1T_e = nc.declare_dram_parameter("wm1T", [H, MH], BF16, isOutput=False)
    wm2T_e = nc.declare_dram_parameter("wm2T", [MH, H], BF16, isOutput=False)
    biasp_e = nc.declare_dram_parameter("biasp", [H, 2], FP32, isOutput=False)
    # out: [128 batch-chunk partitions, T*K*4chunks*2coords] fp32
    NOUT = T * K * 4 * C
    out_e = nc.declare_dram_parameter("out", [128, NOUT], FP32, isOutput=True)

    with tile.TileContext(nc) as tc:
        with ExitStack() as ctx:
            wts = ctx.enter_context(tc.tile_pool(name="wts", bufs=1))
            big = ctx.enter_context(tc.tile_pool(name="big", bufs=1))
            st = ctx.enter_context(tc.tile_pool(name="st", bufs=1))
            tmp = ctx.enter_context(tc.tile_pool(name="tmp", bufs=tmp_bufs))
            psA = ctx.enter_context(tc.tile_pool(name="psA", bufs=2, space="PSUM"))
            psB = ctx.enter_context(tc.tile_pool(name="psB", bufs=2, space="PSUM"))

            # ---- weights / constants into SBUF
            whhT = wts.tile([H, 4 * H], BF16, tag="whhT")
            nc.sync.dma_start(whhT[:], whhT_e[:])
            woutT = wts.tile([H, C], BF16, tag="woutT")
            nc.sync.dma_start(woutT[:], woutT_e[:])
            wm1T = wts.tile([H, MH], BF16, tag="wm1T")
            nc.sync.dma_start(wm1T[:], wm1T_e[:])
            wm2T = wts.tile([MH, H], BF16, tag="wm2T")
            nc.sync.dma_start(wm2T[:], wm2T_e[:])
            biasp = wts.tile([H, 2], FP32, tag="biasp")
            nc.sync.dma_start(biasp[:], biasp_e[:])

            npack = 4 if use_pack else 1
            nprows = 32 * (npack - 1) + CA
            wih = wts.tile([nprows, 4 * H], BF16, tag="wih")
            xrep = wts.tile([nprows, T * BP], BF16, tag="xrep")
            phT = big.tile([H, K * BP], BF16, tag="phT")
            QK = K * BP // 4
            # first quarter first: MLP(0) waits ~2us, not the full transfer
            nc.sync.dma_start(phT[:, 0:QK], phT_e[:, 0:QK])
            for r in range(npack):
                nc.sync.dma_start(wih[32 * r:32 * r + CA, :], wihT_e[:])
                nc.sync.dma_start(xrep[32 * r:32 * r + CA, 0:BP],
                                  xh_e[:, 0:BP])
            nc.sync.dma_start(phT[:, QK:2 * QK], phT_e[:, QK:2 * QK])
            for r in range(npack):
                nc.sync.dma_start(xrep[32 * r:32 * r + CA, BP:T * BP],
                                  xh_e[:, BP:T * BP])
            nc.sync.dma_start(phT[:, 2 * QK:], phT_e[:, 2 * QK:])

            # SBUF accumulation buffer for the final output
            out_sb = big.tile([128, NOUT], FP32, tag="out_sb")

            # ---- persistent state tiles
            h_t = [st.tile([H, FD], BF16, name=f"h{j}", tag=f"h{j}")
                   for j in range(RT)]
            c_t = [st.tile([H, FD], BF16, name=f"c{j}", tag=f"c{j}")
                   for j in range(RT)]

            # ---- mapping MLP -> h0 (interleaved into the first step's
            # pipeline: MLP(j) shares the psA pool rotation and is emitted
            # two rtiles ahead of the gate matmuls that consume h_t[j])
            def emit_mlp(j):
                # hosted on psB: that pool is idle until the step-0 gates,
                # so interleaving MLP(j) with the loop head costs nothing
                pm = psB.tile([H, 2 * PSB], FP32, tag="pb", name="pm")
                nc.tensor.matmul(pm[0:MH, 0:FD], wm1T[:, 0:MH],
                                 phT[:, j * FD:(j + 1) * FD],
                                 start=True, stop=True)
                a1 = tmp.tile([MH, FD], BF16, tag="a1")
                nc.vector._custom_dve(OP_LRELU, out=a1[:], in0=pm[0:MH, 0:FD],
                                      s1=biasp[0:MH, 0:1], imm2=0.01)
                nc.tensor.matmul(pm[0:H, PSB:PSB + FD], wm2T[:, 0:H], a1[:],
                                 start=True, stop=True)
                nc.scalar.activation(h_t[j][:], pm[0:H, PSB:PSB + FD],
                                     AF.Identity, bias=biasp[:, 1:2], scale=1.0)

            for j in range(min(4, RT)):
                emit_mlp(j)
            mlp_next = [min(4, RT)]

            # gate chunk offsets in whhT / wih cols: i=0, f=1, g=2, o=3
            CH = {"i": 0, "f": 1, "g": 2, "o": 3}

            def gcol(name):
                m = CH[name]
                return slice(m * H, (m + 1) * H)

            def gates_mm(ps, xcols, j, names, slices=None):
                # W_ih-[x;1] preloads (contract=3, row-packed), then W_hh accum
                if slices is None:
                    slices = [slice(0, FD) if r == 0 else slice(PSB, PSB + FD)
                              for r in range(len(names))]
                for gname, sl in zip(names, slices):
                    rr = (CH[gname] if use_pack else 0)
                    nc.tensor.matmul(
                        ps[:, sl],
                        wih[32 * rr:32 * rr + CA, gcol(gname)],
                        xrep[32 * rr:32 * rr + CA, xcols],
                        start=True, stop=False,
                        tile_position=(32 * rr, 0) if use_pack else None,
                    )
                for gname, sl in zip(names, slices):
                    nc.tensor.matmul(ps[:, sl], whhT[:, gcol(gname)],
                                     h_t[j][:], start=False, stop=True)

            # ---- flat software-pipelined main loop over s = tl*RT + j.
            # The [i|g] pa-group runs ONE rtile ahead of the [f|o] pb-group so
            # the PE has ~1.7us of queued work inside the a_fo ACT-evac window
            # (pb is single-buffered; its bank is reusable only after a_fo).
            S = T * repeat * RT
            pa_tiles = {}   # s -> (pa_tile, a_i_tile)
            afo_tiles = {}  # s -> a_fo_tile
            pend_h = []     # s queued for OP_H (lagged h_lag rtiles)
            pend_out = []   # s queued for out-proj (lagged out_lag rtiles)

            def s_tj(s):
                return (s // RT) % T, s % RT

            def emit_pa(s):
                t, j = s_tj(s)
                if pair_mode == "ig_fo":
                    pa = psA.tile([H, 2 * PSB], FP32, tag="pa")  # [i | g]
                    gates_mm(pa, slice(t * BP, (t + 1) * BP), j, "ig")
                    a_i = tmp.tile([H, FD], BF16, tag="ai")
                    nc.scalar.activation(a_i[:], pa[:, 0:FD], AF.Relu,
                                         bias=0.0, scale=1.0)
                    pa_tiles[s] = (pa, a_i[:])
                else:  # gf_io: both halves consumed by DVE straight from psum
                    pa = psA.tile([H, 2 * PSB], FP32, tag="pa")  # [g | f]
                    gates_mm(pa, slice(t * BP, (t + 1) * BP), j, "gf")
                    pa_tiles[s] = (pa, None)

            def emit_h(s):
                t, j = s_tj(s)
                a_o_ap = afo_tiles.pop(s)
                r = nc.vector._custom_dve(OP_H, out=h_t[j][:], in0=a_o_ap,
                                          in1=c_t[j][:], imm2=-1.0)
                r.ins.perf_max = 1   # all-bf16 SBUF -> 2X_1PORT eligible

            def emit_out(s, host_pa, col_off=0, evac=True, evac_cols=8):
                # Transposed out-projection: h 128-batch chunk is stationary,
                # woutT moving -> [128 batch, 2] psum slots. The slots land in
                # the DEAD tail of `host_pa`'s g-bank (already consumed by
                # OP_H-as-t), so no dedicated psum bank is needed and psB can
                # double-buffer. Evac'd to out_sb (8 fp32 per rtile-step).
                t, j = s_tj(s)
                sm = s % (T * RT)
                gend = 2 * PSB if pair_mode == "ig_fo" else PSB
                base = gend - 8 - col_off
                for q in range(4):
                    nc.tensor.matmul(host_pa[:, base + C * q:base + C * (q + 1)],
                                     h_t[j][:, 128 * q:128 * (q + 1)],
                                     woutT[:, 0:C], start=True, stop=True)
                if not evac:
                    return
                elo = base + 8 - evac_cols
                start = (sm + 1) * 8 - evac_cols
                eng = out_evac_eng
                if eng == "alt":
                    eng = "scalar" if (sm // 2) % 2 else "vector"
                if eng == "scalar":
                    nc.scalar.activation(out_sb[:, start:start + evac_cols],
                                         host_pa[:, elo:elo + evac_cols],
                                         AF.Copy, bias=0.0, scale=1.0)
                else:
                    nc.vector.tensor_scalar_add(
                        out_sb[:, start:start + evac_cols],
                        host_pa[:, elo:elo + evac_cols], 0.0)
                # stream completed 512-col out_sb regions to DRAM during the
                # loop so no big DMA sits in the drain tail (last repeat only)
                if s >= S - T * RT:
                    end = start + evac_cols
                    if end % 512 == 0 or end == NOUT:
                        lo = (end - 1) // 512 * 512
                        nc.sync.dma_start(out_e[:, lo:end], out_sb[:, lo:end])

            emit_pa(0)
            prev_pa = None
            for s in range(S):
                t, j = s_tj(s)
                if s % 2 == 0 and mlp_next[0] < RT:
                    # drain the MLP in pairs on even iterations: psB then
                    # keeps its buffer parity, so consecutive pb tiles stay
                    # on opposite banks (pairs ahead of the consuming gates)
                    for _ in range(2):
                        if mlp_next[0] < RT:
                            emit_mlp(mlp_next[0])
                            mlp_next[0] += 1
                if out_pos == "early" and prev_pa is not None \
                        and len(pend_out) > out_lag:
                    emit_out(pend_out.pop(0), prev_pa)
                pb = psB.tile([H, 2 * PSB], FP32, tag="pb")
                if pair_mode == "ig_fo":
                    xc = slice(t * BP, (t + 1) * BP)
                    if t == 0:   # f gate (and its 2 matmuls) dead when c==0
                        gates_mm(pb, xc, j, "o",
                                 slices=[slice(PSB, PSB + FD)])
                    else:
                        gates_mm(pb, xc, j, "fo")
                    branch = (("o_only" if s % 2 else "fo_merged")
                              if evac_mode == "alt" else evac_mode)
                    if t == 0:
                        branch = "o_only"   # f gate unused when c==0
                    if branch == "o_only":
                        a_o = tmp.tile([H, FD], BF16, tag="ao")
                        nc.scalar.activation(a_o[:], pb[:, PSB:PSB + FD],
                                             AF.Relu, bias=0.0, scale=1.0)
                        afo_tiles[s] = a_o[:]
                        if t > 0:
                            u_d = tmp.tile([H, FD], BF16, tag="uu")
                            nc.vector._custom_dve(OP_UF, out=u_d[:],
                                                  in0=pb[:, 0:FD],
                                                  in1=c_t[j][:])
                    else:
                        a_fo = tmp.tile([H, 2 * PSB], BF16, tag="afo")
                        nc.scalar.activation(a_fo[:], pb[:, 0:2 * PSB],
                                             AF.Relu, bias=0.0, scale=1.0)
                        afo_tiles[s] = a_fo[:, PSB:PSB + FD]
                        if t > 0:
                            u_d = tmp.tile([H, FD], BF16, tag="uu")
                            r = nc.vector._custom_dve(OP_UC, out=u_d[:],
                                                      in0=a_fo[:, 0:FD],
                                                      in1=c_t[j][:])
                            r.ins.perf_max = 1
                else:  # gf_io: pb = [i | o], one merged ACT evac covers both
                    gates_mm(pb, slice(t * BP, (t + 1) * BP), j, "io")
                    a_io = tmp.tile([H, 2 * PSB], BF16, tag="afo")
                    nc.scalar.activation(a_io[:], pb[:, 0:2 * PSB], AF.Relu,
                                         bias=0.0, scale=1.0)
                    afo_tiles[s] = a_io[:, PSB:PSB + FD]   # a_o
                if s + 1 < S:
                    emit_pa(s + 1)   # prefetched pa pair for the next rtile

                # DVE: t = min(a_i,1)*ht(g); g raw+b read from psum.
                # The g-bank tail (last 8 cols) doubles as the out-proj slot
                # region after this read (WAR-sem'd by the tile pool).
                pa, a_i = pa_tiles.pop(s)
                if pair_mode == "gf_io":
                    a_i = a_io[:, 0:FD]
                    g_ap = pa[:, 0:FD]
                    if t > 0:
                        u_d = tmp.tile([H, FD], BF16, tag="uu")
                        nc.vector._custom_dve(OP_UF, out=u_d[:],
                                              in0=pa[:, PSB:PSB + FD],
                                              in1=c_t[j][:])
                else:
                    g_ap = pa[:, PSB:PSB + FD]
                if t == 0:
                    t_dst = c_t[j]      # c0 = 0 -> c1 = t
                else:
                    t_dst = tmp.tile([H, FD], BF16, tag="tt")
                nc.vector._custom_dve(OP_H, out=t_dst[:], in0=a_i,
                                      in1=g_ap, imm2=-1.0)
                # lagged OP_H (waits out the GPSIMD add latency off-stream)
                if len(pend_h) >= h_lag:
                    emit_h(pend_h.pop(0))
                if t > 0:
                    add_e = getattr(nc, c_add_eng)
                    add_e.tensor_add(c_t[j][:], u_d[:], t_dst[:])
                pend_h.append(s)
                if j == RT - 1:      # step boundary: drain so h is final
                    while pend_h:
                        emit_h(pend_h.pop(0))

                pend_out.append(s)
                if out_pos == "late" and len(pend_out) > out_lag:
                    emit_out(pend_out.pop(0), pa)
                elif out_pos == "batch4" and len(pend_out) > out_lag + 3 \
                        and s % 4 == 3:
                    sms = [p % (T * RT) for p in pend_out[:4]]
                    if sms[0] + 3 == sms[3]:
                        for k in range(3):
                            emit_out(pend_out.pop(0), pa, col_off=8 * (3 - k),
                                     evac=False)
                        emit_out(pend_out.pop(0), pa, col_off=0, evac=True,
                                 evac_cols=32)
                    else:
                        for k in range(4):
                            emit_out(pend_out.pop(0), pa, col_off=8 * (3 - k))
                elif out_pos == "batch2" and len(pend_out) > out_lag + 1 \
                        and s % 2 == 1:
                    # two lagged out-projs share one host tile + one evac
                    s0, s1 = pend_out[0], pend_out[1]
                    if s0 % (T * RT) + 1 == s1 % (T * RT):
                        emit_out(pend_out.pop(0), pa, col_off=8, evac=False)
                        emit_out(pend_out.pop(0), pa, col_off=0, evac=True,
                                 evac_cols=16)
                    else:   # repeat-boundary wrap: evac singly
                        emit_out(pend_out.pop(0), pa, col_off=8)
                        emit_out(pend_out.pop(0), pa, col_off=0)
                prev_pa = pa
            last_pa = pa
            for k in range(len(pend_out)):
                emit_out(pend_out[k], last_pa, col_off=8 * (k + 1))
            pend_out.clear()

    nc.finalize()
    return nc


# ---------------------------------------------------------------- host side
def _bf16(x):
    return np.ascontiguousarray(x, dtype=np.float32).astype(ml_dtypes.bfloat16)


def prep_core_inputs(inputs, core, T, K, BP):
    H, MH, C = H_DIM, MH_DIM, C_DIM
    b0 = core * BP
    ph = np.asarray(inputs["pred_lstm_hidden"], np.float32)[:, b0:b0 + BP, :]
    phT = ph.transpose(2, 0, 1).reshape(H, K * BP)
    idx = np.concatenate([[0], np.arange(T - 1)])
    obs = np.asarray(inputs["obs_traj_rel"], np.float32)
    xs = obs[idx][:, b0:b0 + BP, :C]
    xh = xs.transpose(2, 0, 1).reshape(C, T * BP)
    xh = np.concatenate([xh, np.ones((1, T * BP), np.float32)], axis=0)
    bsum = (np.asarray(inputs["b_ih"], np.float32)
            + np.asarray(inputs["b_hh"], np.float32))
    # per-gate-chunk scale and bias folded into W_hh / W_ih / the x=1 row:
    #   i, f, o chunks: psum = (raw + b)/6 + 0.5 ; g chunk: psum = raw + b
    scale = np.ones(4 * H, np.float32) / 6.0
    scale[2 * H:3 * H] = 1.0
    bias_row = bsum * scale
    bias_row[0:2 * H] += 0.5
    bias_row[3 * H:4 * H] += 0.5
    whh_s = np.asarray(inputs["W_hh"], np.float32) * scale[:, None]
    wih_s = np.asarray(inputs["W_ih"], np.float32) * scale[:, None]
    wih_aug = np.concatenate([wih_s.T, bias_row[None, :]], axis=0)  # [3, 4H]
    biasp = np.zeros((H, 2), np.float32)
    biasp[0:MH, 0] = np.asarray(inputs["bm1"], np.float32)
    biasp[:, 1] = np.asarray(inputs["bm2"], np.float32)
    return {
        "phT": _bf16(phT),
        "xh": _bf16(xh),
        "whhT": _bf16(whh_s.T),
        "wihT": _bf16(wih_aug),
        "woutT": _bf16(np.asarray(inputs["W_out"], np.float32).T),
        "wm1T": _bf16(np.asarray(inputs["Wm1"], np.float32).T),
        "wm2T": _bf16(np.asarray(inputs["Wm2"], np.float32).T),
        "biasp": biasp,
    }


_NC_CACHE = {}


def _get_nc(T, K, BP):
    key = (T, K, BP)
    if key not in _NC_CACHE:
        _NC_CACHE[key] = build_nc(T, K, BP)
    return _NC_CACHE[key]


def kernel(**inputs) -> np.ndarray:
    T, K, B, C = T_FULL, K_FULL, B_FULL, C_DIM
    BP = B // N_CORES
    nc = _get_nc(T, K, BP)
    in_maps = [prep_core_inputs(inputs, c, T, K, BP) for c in range(N_CORES)]
    trace = bool(int(os.environ.get("KERNEL_TRACE", "0")))
    res = run_bass_kernel_spmd(nc, in_maps, list(range(N_CORES)), trace=trace)
    if trace:
        kernel.last_exec_time_ns = res.exec_time_ns
        kernel.last_results = res
    # per-core out: [128, T*K*4*C] -> [T, K, BP, C]
    parts = []
    for c in range(N_CORES):
        arr = res.results[c]["out"].reshape(128, T, K, 4, C)
        parts.append(arr.transpose(1, 2, 3, 0, 4).reshape(T, K, BP, C))
    full = np.concatenate(parts, axis=2)  # [T, K, B, C]
    b_out = np.asarray(inputs["b_out"], np.float32)
    return np.ascontiguousarray(full + b_out, dtype=np.float32)
